# revision 1
# baseline (speedup 1.0000x reference)
"""Multi-head attention on 8 Trainium2 NeuronCores (tensor-parallel over heads).

B=4, S=2048, D=1024, H=16 heads of DK=64. Each core owns 2 heads (a
128-channel slice of the QKV projections). Per core, per batch b:
  xT   = transpose(x[b])           [d=128 x 8, S]  (DMA transpose, bf16)
  QT   = (Wq_c)^T x^T + bq_c       [128, S]        (channels on partitions)
  KT   = (Wk_c)^T x^T + bk_c       [128, S]
  V    = x Wv_c + bv_c             [S, 128] stored per-head with a ones col
  per head h, per q-pair qp (1024 q cols), accumulate over k-chunks kc:
    scT = K Q^T            [k=128, q=1024] psum (2 matmuls)
    ex  = exp(scT / 8)     bf16 (one wide activation)
    av += V_aug^T ex       [65, 512] x2 psum; rows 0-63 ctx^T, row 64 sumexp
  ctxT = av[0:64] * recip(av[64])  (recip broadcast via DRAM bounce)
  out[b] partial = ctx^T Wo_c      [S, D] fp32  (host sums partials + bo)

Matmul inputs are bf16 (1 cyc/col on PE); accumulation is fp32 in PSUM;
softmax stats and the output are fp32. The emission is software-pipelined:
batch b's attention (ACT-heavy) is interleaved with batch b+1's projections
and batch b-1's output projection (PE/DMA-heavy) so all engines stay fed.
"""

import numpy as np

B, S, D, H, DK = 4, 2048, 1024, 16, 64
NCORES = 8
CS = D // NCORES  # 128 channels (2 heads) per core
NSB = S // 128    # 16 s-blocks
NST = S // 512    # 4 s-tiles
NDC = D // 128    # 8 d-chunks

DTYPE = "bf16"  # "bf16" | "fp32"
TRACE = False
LAST_RESULTS = None
_CACHE = {}


def _interleave(main, fill, start_frac=0.2):
    """Spread fill units evenly between main units (order preserved).
    No fill before start_frac of main has been emitted: the engines run
    in static order, so a fill unit whose inputs aren't ready yet would
    stall them."""
    out = []
    fi = 0
    n0 = int(len(main) * start_frac)
    for i, u in enumerate(main):
        out.append(u)
        if i < n0:
            continue
        want = (i - n0 + 1) * len(fill) // max(1, len(main) - n0)
        while fi < want:
            out.append(fill[fi])
            fi += 1
    out.extend(fill[fi:])
    return out


def _build(repeat=1, bench_io=False, dtype=DTYPE):
    import concourse.bass as bass  # noqa: F401
    import concourse.mybir as mybir
    import concourse.tile as tile
    from concourse import bacc
    from concourse.masks import make_identity

    fp32 = mybir.dt.float32
    cdt = mybir.dt.bfloat16 if dtype == "bf16" else fp32
    AF = mybir.ActivationFunctionType

    nc = bacc.Bacc(None, target_bir_lowering=False)
    if bench_io:
        # timing variant: big tensors stay on-device (garbage contents), tiny
        # external I/O so per-call tunnel transfers don't mask exec time
        x_d = nc.dram_tensor("xint", [B, S, D], cdt)
        out_d = nc.dram_tensor("outint", [B, S, D], fp32)
        xin_d = nc.declare_dram_parameter("xin", [128, 128], fp32, isOutput=False)
        xout_d = nc.declare_dram_parameter("xout", [128, 128], fp32, isOutput=True)
    else:
        x_d = nc.declare_dram_parameter("x", [B, S, D], cdt, isOutput=False)
        out_d = nc.declare_dram_parameter("out", [B, S, D], fp32, isOutput=True)
    wq_d = nc.declare_dram_parameter("wq", [D, CS], cdt, isOutput=False)
    wk_d = nc.declare_dram_parameter("wk", [D, CS], cdt, isOutput=False)
    wv_d = nc.declare_dram_parameter("wv", [D, CS], cdt, isOutput=False)
    wo_d = nc.declare_dram_parameter("wo", [CS, D], cdt, isOutput=False)
    bq_d = nc.declare_dram_parameter("bq", [CS], fp32, isOutput=False)
    bk_d = nc.declare_dram_parameter("bk", [CS], fp32, isOutput=False)
    bv_d = nc.declare_dram_parameter("bv", [CS], fp32, isOutput=False)

    with tile.TileContext(nc) as tc:
        with (
            tc.tile_pool(name="consts", bufs=1) as consts,
            tc.tile_pool(name="xt", bufs=2) as xt_pool,
            tc.tile_pool(name="xload", bufs=3) as xload,
            tc.tile_pool(name="qk", bufs=2) as qk_pool,
            tc.tile_pool(name="vp", bufs=2) as v_pool,
            tc.tile_pool(name="exp", bufs=6) as exp_pool,
            tc.tile_pool(name="ctx", bufs=2) as ctx_pool,
            tc.tile_pool(name="avs", bufs=6) as avs_pool,
            tc.tile_pool(name="rec", bufs=4) as rec_pool,
            tc.tile_pool(name="rb", bufs=4) as rb_pool,
            tc.tile_pool(name="outp", bufs=4) as out_pool,
            tc.tile_pool(name="drp", bufs=8, space="DRAM") as dram_pool,
            tc.tile_pool(name="ps1024", bufs=2, space="PSUM") as ps1024,
            tc.tile_pool(name="ps512", bufs=2, space="PSUM") as ps512,
            tc.tile_pool(name="psav", bufs=1, space="PSUM") as psav,
        ):
            # ---- constants (tiles now, loads deferred until after the
            # first x-transpose DMAs are queued) ----
            wq_t = consts.tile([128, NDC, CS], cdt, tag="wq")
            wk_t = consts.tile([128, NDC, CS], cdt, tag="wk")
            wv_t = consts.tile([128, NDC, CS], cdt, tag="wv")
            wo_t = consts.tile([128, D], cdt, tag="wo")
            bq_t = consts.tile([128, 1], fp32, tag="bq")
            bk_t = consts.tile([128, 1], fp32, tag="bk")
            ones_r = consts.tile([128, 64], fp32, tag="ones_r")
            bv_b = consts.tile([128, CS], fp32, tag="bvb")
            if dtype != "bf16":
                ident = consts.tile([128, 128], cdt, tag="ident")

            def load_consts():
                nc.sync.dma_start(
                    wq_t[:], wq_d[:].rearrange("(c p) m -> p c m", p=128)
                )
                nc.sync.dma_start(
                    wk_t[:], wk_d[:].rearrange("(c p) m -> p c m", p=128)
                )
                nc.sync.dma_start(
                    wv_t[:], wv_d[:].rearrange("(c p) m -> p c m", p=128)
                )
                nc.sync.dma_start(wo_t[:], wo_d[:])
                nc.sync.dma_start(bq_t[:], bq_d[:].rearrange("(p o) -> p o", o=1))
                nc.sync.dma_start(bk_t[:], bk_d[:].rearrange("(p o) -> p o", o=1))
                nc.gpsimd.memset(ones_r[:], 1.0)
                nc.sync.dma_start(
                    bv_b[:],
                    bv_d[:].rearrange("(o f) -> o f", o=1).partition_broadcast(128),
                )
                if dtype != "bf16":
                    make_identity(nc, ident[:])
                if bench_io:
                    tio = consts.tile([128, 128], fp32, tag="tio")
                    nc.sync.dma_start(tio[:], xin_d[:])
                    nc.sync.dma_start(xout_d[:], tio[:])

            state = {}

            def A_units(bi, b):
                """x transpose + QKV projections for batch index bi."""
                xT = xt_pool.tile([128, NDC, S], cdt, tag="xT")
                QT = qk_pool.tile([128, S], cdt, tag="QT")
                KT = qk_pool.tile([128, S], cdt, tag="KT")
                v0 = v_pool.tile([128, NSB, 65], cdt, tag="v0")
                v1 = v_pool.tile([128, NSB, 65], cdt, tag="v1")
                state[bi] = dict(xT=xT, QT=QT, KT=KT, v0=v0, v1=v1)
                units = []
                if dtype == "bf16":
                    xr = x_d[b].rearrange("M (c p) -> M c p", p=128)
                    for cch in range(NDC):
                        units.append(
                            lambda cch=cch: nc.sync.dma_start(
                                xT[:, cch, :], xr[:, cch], transpose=True
                            )
                        )
                else:
                    for sb in range(NSB):
                        def u_x(sb=sb):
                            xl = xload.tile([128, D], cdt, tag="xl")
                            nc.sync.dma_start(
                                xl[:], x_d[b, sb * 128 : (sb + 1) * 128, :]
                            )
                            for cch in range(NDC):
                                pt = ps512.tile([128, 128], fp32, tag="mm512")
                                nc.tensor.transpose(
                                    pt[:], xl[:, cch * 128 : (cch + 1) * 128],
                                    ident[:],
                                )
                                nc.vector.tensor_copy(
                                    xT[:, cch, sb * 128 : (sb + 1) * 128], pt[:]
                                )
                        units.append(u_x)

                for st in range(NST):
                    def u_q(st=st):
                        sl = slice(st * 512, (st + 1) * 512)
                        pq = ps512.tile([128, 512], fp32, tag="mm512")
                        for cch in range(NDC):
                            nc.tensor.matmul(
                                pq[:], wq_t[:, cch, :], xT[:, cch, sl],
                                start=(cch == 0), stop=(cch == NDC - 1),
                            )
                        nc.vector.tensor_scalar_add(QT[:, sl], pq[:], bq_t[:])
                    units.append(u_q)

                    def u_k(st=st):
                        sl = slice(st * 512, (st + 1) * 512)
                        pk = ps512.tile([128, 512], fp32, tag="mm512")
                        for cch in range(NDC):
                            nc.tensor.matmul(
                                pk[:], wk_t[:, cch, :], xT[:, cch, sl],
                                start=(cch == 0), stop=(cch == NDC - 1),
                            )
                        nc.vector.tensor_scalar_add(KT[:, sl], pk[:], bk_t[:])
                    units.append(u_k)

                def u_ones():
                    nc.gpsimd.memset(v0[:, :, 64:65], 1.0)
                    nc.gpsimd.memset(v1[:, :, 64:65], 1.0)
                units.append(u_ones)

                for sb in range(NSB):
                    def u_v(sb=sb):
                        pv = ps512.tile([128, 128], fp32, tag="mm512")
                        for cch in range(NDC):
                            nc.tensor.matmul(
                                pv[:], xT[:, cch, sb * 128 : (sb + 1) * 128],
                                wv_t[:, cch, :],
                                start=(cch == 0), stop=(cch == NDC - 1),
                            )
                        nc.vector.tensor_add(
                            v0[:, sb, 0:64], pv[:, 0:64], bv_b[:, 0:64]
                        )
                        nc.vector.tensor_add(
                            v1[:, sb, 0:64], pv[:, 64:128], bv_b[:, 64:128]
                        )
                    units.append(u_v)
                return units

            def B_units(bi):
                st_ = state[bi]
                QT, KT, v0, v1 = st_["QT"], st_["KT"], st_["v0"], st_["v1"]
                ctx0 = ctx_pool.tile([128, S // 2], cdt, tag="ctx0")
                ctx1 = ctx_pool.tile([128, S // 2], cdt, tag="ctx1")
                st_["ctx"] = (ctx0, ctx1)
                sections = []
                for qp in range(NST // 2):
                    q0 = qp * 1024
                    ctxq = (ctx0, ctx1)[qp]
                    units = []
                    for h in (0, 1):
                        hoff = 64 * h
                        vh = v0 if h == 0 else v1
                        avpair = []

                        def u_alloc(avpair=avpair):
                            av0_t = psav.tile([65, 512], fp32, tag="av0")
                            avpair.append(av0_t)
                            av1_t = psav.tile([65, 512], fp32, tag="av1")
                            avpair.append(av1_t)
                        units.append(u_alloc)

                        exq = []

                        def u_sc(hoff=hoff, q0=q0, kc=0, exq=exq):
                            ksl = slice(kc * 128, (kc + 1) * 128)
                            sc = ps1024.tile([128, 1024], fp32, tag="mm1024")
                            nc.tensor.matmul(
                                sc[:, 0:512],
                                KT[hoff : hoff + 64, ksl],
                                QT[hoff : hoff + 64, q0 : q0 + 512],
                                start=True, stop=True,
                            )
                            nc.tensor.matmul(
                                sc[:, 512:1024],
                                KT[hoff : hoff + 64, ksl],
                                QT[hoff : hoff + 64, q0 + 512 : q0 + 1024],
                                start=True, stop=True,
                            )
                            ex = exp_pool.tile([128, 1024], cdt, tag="ex")
                            nc.scalar.activation(ex[:], sc[:], AF.Exp, scale=0.125)
                            exq.append(ex)

                        def u_av(vh=vh, kc=0, avpair=avpair, exq=exq):
                            ex = exq[kc]
                            for sub in (0, 1):
                                nc.tensor.matmul(
                                    avpair[sub][:],
                                    vh[:, kc, :],
                                    ex[:, sub * 512 : (sub + 1) * 512],
                                    start=(kc == 0), stop=(kc == NSB - 1),
                                    skip_group_check=True,
                                )

                        from functools import partial
                        for kc in range(NSB):
                            def u_kc(kc=kc, u_sc=u_sc, u_av=u_av):
                                u_sc(kc=kc)
                                if kc > 0:
                                    u_av(kc=kc - 1)
                                if kc == NSB - 1:
                                    u_av(kc=kc)
                            units.append(u_kc)

                        def u_norm(hoff=hoff, ctxq=ctxq, avpair=avpair):
                            for sub in (0, 1):
                                avx = avpair[sub]
                                qsl = slice(sub * 512, (sub + 1) * 512)
                                # free the psum bank fast, then normalize
                                av_s = avs_pool.tile([65, 512], fp32, tag="avs")
                                nc.vector.tensor_copy(av_s[:], avx[:])
                                rec = rec_pool.tile([65, 512], fp32, tag="rec")
                                nc.vector.reciprocal(rec[64:65, :], av_s[64:65, :])
                                dr = dram_pool.tile([1, 512], fp32, tag="dr")
                                nc.sync.dma_start(dr[:], rec[64:65, :])
                                rb = rb_pool.tile([64, 512], fp32, tag="rb")
                                nc.sync.dma_start(rb[:], dr[:].partition_broadcast(64))
                                nc.vector.tensor_mul(
                                    ctxq[hoff : hoff + 64, qsl], av_s[0:64, :], rb[:]
                                )
                        units.append(u_norm)
                    sections.append(units)
                return sections

            def C_units(bi, b):
                st_ = state[bi]
                ctx = st_["ctx"]
                halves = ([], [])
                for sb in range(NSB):
                    def u_o(sb=sb):
                        ctxq = ctx[sb // 8]
                        lsl = slice((sb % 8) * 128, (sb % 8 + 1) * 128)
                        ot = out_pool.tile([128, D], fp32, tag="ot")
                        for half in range(2):
                            osl = slice(half * 512, (half + 1) * 512)
                            po = ps512.tile([128, 512], fp32, tag="mm512")
                            nc.tensor.matmul(
                                po[:], ctxq[:, lsl], wo_t[:, osl],
                                start=True, stop=True,
                            )
                            nc.vector.tensor_copy(ot[:, osl], po[:])
                        nc.sync.dma_start(
                            out_d[b, sb * 128 : (sb + 1) * 128, :], ot[:]
                        )
                    halves[sb // 8].append(u_o)
                return halves

            bs = [bb for _ in range(repeat) for bb in range(B)]
            n_x_units = NDC if dtype == "bf16" else NSB
            a0 = A_units(0, bs[0])
            for u in a0[:n_x_units]:
                u()
            load_consts()
            for u in a0[n_x_units:]:
                u()
            c_tail = []
            for bi, b in enumerate(bs):
                sec0, sec1 = B_units(bi)
                if bi + 1 < len(bs):
                    a_next = A_units(bi + 1, bs[bi + 1])
                    xdmas, a_rest = a_next[:n_x_units], a_next[n_x_units:]
                else:
                    xdmas, a_rest = [], []
                # start next batch's x transposes immediately
                for u in xdmas:
                    u()
                half = len(a_rest) // 2
                for u in _interleave(sec0, c_tail + a_rest[:half]):
                    u()
                c_head, new_tail = C_units(bi, b)
                for u in _interleave(sec1, c_head + a_rest[half:]):
                    u()
                c_tail = new_tail
                del state[bi]
            for u in c_tail:
                u()

    nc.compile()
    return nc


def _get_nc(repeat=1, bench_io=False, dtype=None):
    if dtype is None:
        dtype = DTYPE
    key = f"nc{repeat}_{bench_io}_{dtype}"
    if key not in _CACHE:
        _CACHE[key] = _build(repeat, bench_io, dtype)
    return _CACHE[key]


def kernel(**inputs):
    global LAST_RESULTS
    import ml_dtypes
    from concourse.bass_utils import run_bass_kernel_spmd

    cdt = ml_dtypes.bfloat16 if DTYPE == "bf16" else np.float32
    x = np.ascontiguousarray(np.asarray(inputs["x"], dtype=np.float32).astype(cdt))
    Wq = np.asarray(inputs["Wq"], dtype=np.float32).astype(cdt)
    Wk = np.asarray(inputs["Wk"], dtype=np.float32).astype(cdt)
    Wv = np.asarray(inputs["Wv"], dtype=np.float32).astype(cdt)
    Wo = np.asarray(inputs["Wo"], dtype=np.float32).astype(cdt)
    bq = np.asarray(inputs["bq"], dtype=np.float32)
    bk = np.asarray(inputs["bk"], dtype=np.float32)
    bv = np.asarray(inputs["bv"], dtype=np.float32)
    bo = np.asarray(inputs["bo"], dtype=np.float32)

    nc = _get_nc()
    in_maps = []
    for c in range(NCORES):
        cs = slice(CS * c, CS * (c + 1))
        in_maps.append(
            {
                "x": x,
                "wq": np.ascontiguousarray(Wq[:, cs]),
                "wk": np.ascontiguousarray(Wk[:, cs]),
                "wv": np.ascontiguousarray(Wv[:, cs]),
                "wo": np.ascontiguousarray(Wo[cs, :]),
                "bq": np.ascontiguousarray(bq[cs]),
                "bk": np.ascontiguousarray(bk[cs]),
                "bv": np.ascontiguousarray(bv[cs]),
            }
        )
    res = run_bass_kernel_spmd(
        nc, in_maps, core_ids=list(range(NCORES)), trace=TRACE
    )
    LAST_RESULTS = res
    acc = np.zeros((B, S, D), dtype=np.float64)
    for c in range(NCORES):
        acc += res.results[c]["out"]
    acc += bo
    return acc.astype(np.float32)



# revision 6
# speedup vs baseline: 1.0854x; 1.0854x over previous
"""Multi-head attention on 8 Trainium2 NeuronCores (tensor-parallel over heads).

B=4, S=2048, D=1024, H=16 heads of DK=64. Each core owns 2 heads (a
128-channel slice of the QKV projections). x is pre-transposed on the
host to [B, D, S] so the device reads contiguous rows (no DMA transpose).

Per core, per batch b (all matmul operands bf16, fp32 PSUM accumulate):
  QT   = Wq_c^T xT + bq    [128, S]   (both heads stacked on partitions)
  KT   = Wk_c^T xT         [128, S]   (bk cancels in softmax -- dropped)
  V    = xT^T Wv_c         [S, 128] stored per-head [128, 16, 64]
  attention per (q-pair, 512-wide sub), accumulating over k-chunks kc:
    sc[128, 2, 512] psum = K_h Q_h^T for h0, h1 -- the two C=64 matmuls
       are issued adjacently so they run concurrently in the PE array
       (row tiling at partitions 0/64); sc double-buffered so ACT and PE
       ping-pong without stalls
    ex = exp(sc/8)  one ACT instr, N=1024 (amortizes the 352-cyc overhead)
    av[128, 512] psum += [V0^T ex0 ; V1^T ex1]  (col-tiled pair, M=64
       each at col positions 0/64 -- ctx^T lands in O-projection layout)
    se psum += ones^T ex   (2 col-tiled M=1 matmuls; rows 0/32 for sub0,
       64/96 for sub1, one se bank per q-pair)
  rse = 1/se (one DVE reciprocal per q-pair), broadcast via DRAM bounce,
  ctx = av * rse on DVE, out partial = ctx^T Wo_c  [S, D] bf16.
Host sums the 8 cores' partials and adds bo + bv@Wo (bv commutes through
softmax since the attention weights sum to 1).
"""

import numpy as np

B, S, D, H, DK = 4, 2048, 1024, 16, 64
NCORES = 8
CS = D // NCORES   # 128 channels (2 heads) per core
NDC = D // 128     # 8 d-chunks
NKC = S // 128     # 16 k-chunks
NST = S // 512     # 4 s-tiles
NQP = S // 1024    # 2 q-pairs

TRACE = False
LAST_RESULTS = None
_CACHE = {}


def _interleave(main, fill, start_frac=0.2):
    """Spread fill units evenly between main units (order preserved).
    No fill before start_frac of main has been emitted: the engines run
    in static order, so a fill unit whose inputs aren't ready yet would
    stall them."""
    out = []
    fi = 0
    n0 = int(len(main) * start_frac)
    for i, u in enumerate(main):
        out.append(u)
        if i < n0:
            continue
        want = (i - n0 + 1) * len(fill) // max(1, len(main) - n0)
        while fi < want:
            out.append(fill[fi])
            fi += 1
    out.extend(fill[fi:])
    return out


def _build():
    import concourse.bass as bass  # noqa: F401
    import concourse.mybir as mybir
    import concourse.tile as tile
    from concourse import bacc

    fp32 = mybir.dt.float32
    bf16 = mybir.dt.bfloat16
    AF = mybir.ActivationFunctionType

    nc = bacc.Bacc(None, target_bir_lowering=False)
    xt_d = nc.declare_dram_parameter("xt", [B, D, S], bf16, isOutput=False)
    out_d = nc.declare_dram_parameter("out", [B, S, D], bf16, isOutput=True)
    wq_d = nc.declare_dram_parameter("wq", [D, CS], bf16, isOutput=False)
    wk_d = nc.declare_dram_parameter("wk", [D, CS], bf16, isOutput=False)
    wv_d = nc.declare_dram_parameter("wv", [D, CS], bf16, isOutput=False)
    wo_d = nc.declare_dram_parameter("wo", [CS, D], bf16, isOutput=False)
    bq_d = nc.declare_dram_parameter("bq", [CS], fp32, isOutput=False)

    with tile.TileContext(nc) as tc:
        with (
            tc.tile_pool(name="consts", bufs=1) as consts,
            tc.tile_pool(name="xt", bufs=2) as xt_pool,
            tc.tile_pool(name="qk", bufs=2) as qk_pool,
            tc.tile_pool(name="vp", bufs=2) as v_pool,
            tc.tile_pool(name="exq", bufs=3) as ex_pool,
            tc.tile_pool(name="ctx", bufs=2) as ctx_pool,
            tc.tile_pool(name="avs", bufs=4) as avs_pool,
            tc.tile_pool(name="rec", bufs=2) as rec_pool,
            tc.tile_pool(name="rb", bufs=8) as rb_pool,
            tc.tile_pool(name="outp", bufs=4) as out_pool,
            tc.tile_pool(name="drp", bufs=8, space="DRAM") as dram_pool,
            tc.tile_pool(name="pssc", bufs=2, space="PSUM") as ps_sc,
            tc.tile_pool(name="psav", bufs=2, space="PSUM") as ps_av,
            tc.tile_pool(name="psse", bufs=1, space="PSUM") as ps_se,
            tc.tile_pool(name="pspj", bufs=1, space="PSUM") as ps_pj,
        ):
            wq_t = consts.tile([128, NDC, CS], bf16, tag="wq")
            wk_t = consts.tile([128, NDC, CS], bf16, tag="wk")
            wv_t = consts.tile([128, NDC, CS], bf16, tag="wv")
            wo_t = consts.tile([128, D], bf16, tag="wo")
            bq_t = consts.tile([128, 1], fp32, tag="bq")
            ones_t = consts.tile([128, 1], bf16, tag="ones")

            def load_consts():
                nc.sync.dma_start(
                    wq_t[:], wq_d[:].rearrange("(c p) m -> p c m", p=128)
                )
                nc.sync.dma_start(
                    wk_t[:], wk_d[:].rearrange("(c p) m -> p c m", p=128)
                )
                nc.sync.dma_start(
                    wv_t[:], wv_d[:].rearrange("(c p) m -> p c m", p=128)
                )
                nc.sync.dma_start(wo_t[:], wo_d[:])
                nc.sync.dma_start(bq_t[:], bq_d[:].rearrange("(p o) -> p o", o=1))
                nc.gpsimd.memset(ones_t[:], 1.0)

            state = {}

            def A_xdma(bi, b):
                xT = xt_pool.tile([128, NDC, S], bf16, tag="xT")
                state[bi] = dict(xT=xT)
                xr = xt_d[b].rearrange("(c p) M -> p c M", p=128)
                return [
                    (lambda cch=cch: nc.sync.dma_start(
                        xT[:, cch, :], xr[:, cch]))
                    for cch in range(NDC)
                ]

            def A_units(bi):
                st_ = state[bi]
                xT = st_["xT"]
                QT = qk_pool.tile([128, S], bf16, tag="QT")
                KT = qk_pool.tile([128, S], bf16, tag="KT")
                v0 = v_pool.tile([128, NKC, DK], bf16, tag="v0")
                v1 = v_pool.tile([128, NKC, DK], bf16, tag="v1")
                st_.update(QT=QT, KT=KT, v0=v0, v1=v1)
                units = []
                for st in range(NST):
                    def u_q(st=st):
                        sl = slice(st * 512, (st + 1) * 512)
                        pq = ps_pj.tile([128, 512], fp32, tag="pj")
                        for cch in range(NDC):
                            nc.tensor.matmul(
                                pq[:], wq_t[:, cch, :], xT[:, cch, sl],
                                start=(cch == 0), stop=(cch == NDC - 1),
                            )
                        nc.vector.tensor_scalar_add(QT[:, sl], pq[:], bq_t[:])
                    units.append(u_q)

                    def u_k(st=st):
                        sl = slice(st * 512, (st + 1) * 512)
                        pk = ps_pj.tile([128, 512], fp32, tag="pj")
                        for cch in range(NDC):
                            nc.tensor.matmul(
                                pk[:], wk_t[:, cch, :], xT[:, cch, sl],
                                start=(cch == 0), stop=(cch == NDC - 1),
                            )
                        nc.vector.tensor_copy(KT[:, sl], pk[:])
                    units.append(u_k)

                for g in range(4):  # groups of 4 s-blocks
                    def u_v(g=g):
                        pv = ps_pj.tile([128, 512], fp32, tag="pj")
                        for j in range(4):
                            sb = g * 4 + j
                            qsl = slice(j * 128, (j + 1) * 128)
                            for cch in range(NDC):
                                nc.tensor.matmul(
                                    pv[:, qsl],
                                    xT[:, cch, sb * 128 : (sb + 1) * 128],
                                    wv_t[:, cch, :],
                                    start=(cch == 0), stop=(cch == NDC - 1),
                                    skip_group_check=True,
                                )
                        pvv = pv[:].rearrange("p (j c) -> p j c", j=4)
                        nc.vector.tensor_copy(
                            v0[:, g * 4 : (g + 1) * 4, :], pvv[:, :, 0:DK]
                        )
                        nc.vector.tensor_copy(
                            v1[:, g * 4 : (g + 1) * 4, :], pvv[:, :, DK:CS]
                        )
                    units.append(u_v)
                return units

            def B_units(bi):
                """Attention for batch bi: 4 sections (qp, sub)."""
                st_ = state[bi]
                QT, KT, v0, v1 = st_["QT"], st_["KT"], st_["v0"], st_["v1"]
                ctxs = []
                sections = []
                qp_carry = {}
                for qp in range(NQP):
                    ctx = ctx_pool.tile([128, 1024], bf16, tag="ctx")
                    ctxs.append(ctx)
                    for sub in range(2):
                        q0 = qp * 1024 + sub * 512
                        carry = {}
                        units = []

                        def u_start(carry=carry, sub=sub, qp_carry=qp_carry):
                            av = ps_av.tile([128, 512], fp32, tag="av")
                            carry["av"] = av
                            carry["ex"] = {}
                            if sub == 0:
                                se = ps_se.tile([128, 512], fp32, tag="se")
                                nc.vector.memset(se[:], 1.0)
                                qp_carry["se"] = se
                                qp_carry["avs"] = []
                        units.append(u_start)

                        def u_sc(kc, q0=q0, carry=carry):
                            ksl = slice(kc * 128, (kc + 1) * 128)
                            qsl = slice(q0, q0 + 512)
                            sc = ps_sc.tile([128, 2, 512], fp32, tag="sc")
                            # h0/h1 adjacent -> concurrent row tiles (0/64)
                            nc.tensor.matmul(
                                sc[:, 0, :], KT[0:DK, ksl], QT[0:DK, qsl],
                                start=True, stop=True,
                            )
                            nc.tensor.matmul(
                                sc[:, 1, :], KT[DK:CS, ksl], QT[DK:CS, qsl],
                                start=True, stop=True,
                            )
                            ex = ex_pool.tile([128, 2, 512], bf16, tag="ex")
                            nc.scalar.activation(ex[:], sc[:], AF.Exp, scale=0.125)
                            carry["ex"][kc] = ex

                        def u_av(kc, sub=sub, carry=carry, qp_carry=qp_carry):
                            ex = carry["ex"].pop(kc)
                            av = carry["av"]
                            se = qp_carry["se"]
                            first, last = kc == 0, kc == NKC - 1
                            # col-tiled pair: h0 -> rows 0-63, h1 -> 64-127
                            nc.tensor.matmul(
                                av[0:DK, :], v0[:, kc, :], ex[:, 0, :],
                                start=first, stop=last, skip_group_check=True,
                            )
                            nc.tensor.matmul(
                                av[DK:CS, :], v1[:, kc, :], ex[:, 1, :],
                                start=first, stop=last, skip_group_check=True,
                            )
                            # sumexp rows: sub0 -> 0/32, sub1 -> 64/96
                            for h in range(2):
                                p = 64 * sub + 32 * h
                                nc.tensor.matmul(
                                    se[p : p + 1, :], ones_t[:], ex[:, h, :],
                                    start=first, stop=last,
                                    skip_group_check=True,
                                    tile_position=(0, p),
                                )

                        for kc in range(NKC):
                            def u_kc(kc=kc, u_sc=u_sc, u_av=u_av):
                                u_sc(kc)
                                if kc > 0:
                                    u_av(kc - 1)
                                if kc == NKC - 1:
                                    u_av(kc)
                            units.append(u_kc)

                        def u_end(sub=sub, carry=carry, qp_carry=qp_carry,
                                  ctx=ctx):
                            avs = avs_pool.tile([128, 512], fp32, tag="avs")
                            nc.vector.tensor_copy(avs[:], carry["av"][:])
                            qp_carry["avs"].append(avs)
                            if sub == 0:
                                return
                            # end of q-pair: normalize both subs
                            se = qp_carry["se"]
                            rse = rec_pool.tile([128, 512], fp32, tag="rse")
                            nc.vector.reciprocal(rse[:], se[:])
                            for s in range(2):
                                # rb rows 0-63 <- 1/se(h0), 64-127 <- 1/se(h1)
                                rb = rb_pool.tile([128, 512], fp32, tag="rb")
                                for h in range(2):
                                    dr = dram_pool.tile([1, 512], fp32, tag="dr")
                                    nc.sync.dma_start(
                                        dr[:],
                                        rse[64 * s + 32 * h : 64 * s + 32 * h + 1, :],
                                    )
                                    nc.sync.dma_start(
                                        rb[h * DK : (h + 1) * DK, :],
                                        dr[:].partition_broadcast(DK),
                                    )
                                ssl = slice(s * 512, (s + 1) * 512)
                                avx = qp_carry["avs"][s]
                                nc.vector.tensor_mul(
                                    ctx[:, ssl], avx[:], rb[:]
                                )
                        units.append(u_end)
                        sections.append(units)
                st_["ctx"] = ctxs
                return sections

            def C_units(bi, b, qp):
                ctx = state[bi]["ctx"][qp]
                units = []
                for j in range(8):
                    def u_o(j=j):
                        sb = qp * 8 + j
                        lsl = slice(j * 128, (j + 1) * 128)
                        ot = out_pool.tile([128, D], bf16, tag="ot")
                        for half in range(2):
                            osl = slice(half * 512, (half + 1) * 512)
                            po = ps_pj.tile([128, 512], fp32, tag="pj")
                            nc.tensor.matmul(
                                po[:], ctx[:, lsl], wo_t[:, osl],
                                start=True, stop=True,
                            )
                            nc.vector.tensor_copy(ot[:, osl], po[:])
                        nc.sync.dma_start(
                            out_d[b, sb * 128 : (sb + 1) * 128, :], ot[:]
                        )
                    units.append(u_o)
                return units

            # ---- software pipeline over batches ----
            for u in A_xdma(0, 0):
                u()
            load_consts()
            a_cur = A_units(0)
            c_prev = []  # C units of (bi-1, qp1)
            for bi in range(B):
                for u in a_cur:
                    u()
                secs = B_units(bi)
                if bi + 1 < B:
                    for u in A_xdma(bi + 1, bi + 1):
                        u()
                    a_next = A_units(bi + 1)
                else:
                    a_next = []
                h3 = len(a_next) // 4
                # qp0 attention: filled with prev batch's qp1 out-proj and
                # the start of next batch's projections
                for u in _interleave(secs[0], c_prev[:4] + a_next[:h3], 0.15):
                    u()
                for u in _interleave(secs[1], c_prev[4:] + a_next[h3:2 * h3], 0.15):
                    u()
                c_q0 = C_units(bi, bi, 0)
                for u in _interleave(secs[2], c_q0[:4] + a_next[2 * h3:3 * h3], 0.15):
                    u()
                for u in _interleave(secs[3], c_q0[4:] + a_next[3 * h3:], 0.15):
                    u()
                c_prev = C_units(bi, bi, 1)
                a_cur = []
            for u in c_prev:
                u()

    nc.compile()
    return nc


def _get_nc():
    if "nc" not in _CACHE:
        _CACHE["nc"] = _build()
    return _CACHE["nc"]


def kernel(**inputs):
    global LAST_RESULTS
    import ml_dtypes
    from concourse.bass_utils import run_bass_kernel_spmd

    bf = ml_dtypes.bfloat16
    x = np.asarray(inputs["x"], dtype=np.float32)
    xt = np.ascontiguousarray(x.transpose(0, 2, 1)).astype(bf)  # [B, D, S]
    Wq = np.asarray(inputs["Wq"], dtype=np.float32).astype(bf)
    Wk = np.asarray(inputs["Wk"], dtype=np.float32).astype(bf)
    Wv = np.asarray(inputs["Wv"], dtype=np.float32).astype(bf)
    Wo = np.asarray(inputs["Wo"], dtype=np.float32).astype(bf)
    bq = np.asarray(inputs["bq"], dtype=np.float32)
    bv = np.asarray(inputs["bv"], dtype=np.float32)
    bo = np.asarray(inputs["bo"], dtype=np.float32)

    nc = _get_nc()
    in_maps = []
    for c in range(NCORES):
        cs = slice(CS * c, CS * (c + 1))
        in_maps.append(
            {
                "xt": xt,
                "wq": np.ascontiguousarray(Wq[:, cs]),
                "wk": np.ascontiguousarray(Wk[:, cs]),
                "wv": np.ascontiguousarray(Wv[:, cs]),
                "wo": np.ascontiguousarray(Wo[cs, :]),
                "bq": np.ascontiguousarray(bq[cs]),
            }
        )
    res = run_bass_kernel_spmd(
        nc, in_maps, core_ids=list(range(NCORES)), trace=TRACE
    )
    LAST_RESULTS = res
    acc = np.zeros((B, S, D), dtype=np.float64)
    for c in range(NCORES):
        acc += np.asarray(res.results[c]["out"], dtype=np.float64)
    # bk drops out of softmax; bv commutes through (sum of weights = 1)
    acc += bo + bv.astype(np.float64) @ np.asarray(
        inputs["Wo"], dtype=np.float64
    )
    return acc.astype(np.float32)


# revision 13
# speedup vs baseline: 1.2456x; 1.1476x over previous
"""Multi-head attention on 8 Trainium2 NeuronCores (tensor-parallel over heads).

B=4, S=2048, D=1024, H=16 heads of DK=64. Each core owns 2 heads (a
128-channel slice of the QKV projections). x is pre-transposed on the
host to [B, D, S] so the device reads contiguous rows (no DMA transpose).

Per core, per batch b (all matmul operands bf16, fp32 PSUM accumulate):
  QT   = Wq_c^T xT + bq    [128, S]   (both heads stacked on partitions)
  KT   = Wk_c^T xT         [128, S]   (bk cancels in softmax -- dropped)
  V    = xT^T Wv_c         [S, 128] stored per-head [128, 16, 64]
  attention per (q-pair, 512-wide sub), accumulating over k-chunks kc:
    sc[128, 2, 512] psum = K_h Q_h^T for h0, h1 -- the two C=64 matmuls
       are issued adjacently so they run concurrently in the PE array
       (row tiling at partitions 0/64); sc double-buffered so ACT and PE
       ping-pong without stalls
    ex = exp(sc/8)  one ACT instr, N=1024 (amortizes the 352-cyc overhead)
    av[128, 512] psum += [V0^T ex0 ; V1^T ex1]  (col-tiled pair, M=64
       each at col positions 0/64 -- ctx^T lands in O-projection layout)
    se psum += ones^T ex   (2 col-tiled M=1 matmuls; rows 0/32 for sub0,
       64/96 for sub1, one se bank per q-pair)
  rse = 1/se (one DVE reciprocal per q-pair), broadcast via DRAM bounce,
  ctx = av * rse on DVE, out partial = ctx^T Wo_c  [S, D] bf16.
Host sums the 8 cores' partials and adds bo + bv@Wo (bv commutes through
softmax since the attention weights sum to 1).
"""

import numpy as np

B, S, D, H, DK = 4, 2048, 1024, 16, 64
NCORES = 8
CS = D // NCORES   # 128 channels (2 heads) per core
NDC = D // 128     # 8 d-chunks
NKC = S // 128     # 16 k-chunks
NST = S // 512     # 4 s-tiles
NQP = S // 1024    # 2 q-pairs

TRACE = False
LAST_RESULTS = None
_CACHE = {}


def _interleave(main, fill, start_frac=0.2):
    """Spread fill units evenly between main units (order preserved).
    No fill before start_frac of main has been emitted: the engines run
    in static order, so a fill unit whose inputs aren't ready yet would
    stall them."""
    out = []
    fi = 0
    n0 = int(len(main) * start_frac)
    for i, u in enumerate(main):
        out.append(u)
        if i < n0:
            continue
        want = (i - n0 + 1) * len(fill) // max(1, len(main) - n0)
        while fi < want:
            out.append(fill[fi])
            fi += 1
    out.extend(fill[fi:])
    return out


def _build():
    import concourse.bass as bass  # noqa: F401
    import concourse.mybir as mybir
    import concourse.tile as tile
    from concourse import bacc

    fp32 = mybir.dt.float32
    bf16 = mybir.dt.bfloat16
    AF = mybir.ActivationFunctionType

    nc = bacc.Bacc(None, target_bir_lowering=False)
    xt_d = nc.declare_dram_parameter("xt", [B, D, S], bf16, isOutput=False)
    out_d = nc.declare_dram_parameter("out", [B, S, D], bf16, isOutput=True)
    wq_d = nc.declare_dram_parameter("wq", [D, CS], bf16, isOutput=False)
    wk_d = nc.declare_dram_parameter("wk", [D, CS], bf16, isOutput=False)
    wv_d = nc.declare_dram_parameter("wv", [D, CS], bf16, isOutput=False)
    wo_d = nc.declare_dram_parameter("wo", [CS, D], bf16, isOutput=False)
    bq_d = nc.declare_dram_parameter("bq", [CS], fp32, isOutput=False)

    with tile.TileContext(nc) as tc:
        with (
            tc.tile_pool(name="consts", bufs=1) as consts,
            tc.tile_pool(name="xt", bufs=2) as xt_pool,
            tc.tile_pool(name="qk", bufs=2) as qk_pool,
            tc.tile_pool(name="vp", bufs=2) as v_pool,
            tc.tile_pool(name="exq", bufs=6) as ex_pool,
            tc.tile_pool(name="ctx", bufs=2) as ctx_pool,
            tc.tile_pool(name="avs", bufs=4) as avs_pool,
            tc.tile_pool(name="rec", bufs=2) as rec_pool,
            tc.tile_pool(name="rb", bufs=8) as rb_pool,
            tc.tile_pool(name="outp", bufs=4) as out_pool,
            tc.tile_pool(name="drp", bufs=8, space="DRAM") as dram_pool,
            tc.tile_pool(name="pssc", bufs=2, space="PSUM") as ps_sc,
            tc.tile_pool(name="psav", bufs=2, space="PSUM") as ps_av,
            tc.tile_pool(name="psse", bufs=1, space="PSUM") as ps_se,
            tc.tile_pool(name="pspj", bufs=1, space="PSUM") as ps_pj,
        ):
            wq_t = consts.tile([128, NDC, CS], bf16, tag="wq")
            wk_t = consts.tile([128, NDC, CS], bf16, tag="wk")
            wv_t = consts.tile([128, NDC, CS], bf16, tag="wv")
            wo_t = consts.tile([128, D], bf16, tag="wo")
            bq_t = consts.tile([128, 1], fp32, tag="bq")
            ones_t = consts.tile([128, 1], bf16, tag="ones")

            def load_consts():
                nc.sync.dma_start(
                    wq_t[:], wq_d[:].rearrange("(c p) m -> p c m", p=128)
                )
                nc.sync.dma_start(
                    wk_t[:], wk_d[:].rearrange("(c p) m -> p c m", p=128)
                )
                nc.sync.dma_start(
                    wv_t[:], wv_d[:].rearrange("(c p) m -> p c m", p=128)
                )
                nc.sync.dma_start(wo_t[:], wo_d[:])
                nc.sync.dma_start(bq_t[:], bq_d[:].rearrange("(p o) -> p o", o=1))
                nc.gpsimd.memset(ones_t[:], 1.0)

            state = {}

            def A_xdma(bi, b):
                xT = xt_pool.tile([128, NDC, S], bf16, tag="xT")
                state[bi] = dict(xT=xT)
                xr = xt_d[b].rearrange("(c p) M -> p c M", p=128)
                return [
                    (lambda cch=cch: nc.sync.dma_start(
                        xT[:, cch, :], xr[:, cch]))
                    for cch in range(NDC)
                ]

            def A_units(bi):
                st_ = state[bi]
                xT = st_["xT"]
                QT = qk_pool.tile([128, S], bf16, tag="QT")
                KT = qk_pool.tile([128, S], bf16, tag="KT")
                v0 = v_pool.tile([128, NKC, DK], bf16, tag="v0")
                v1 = v_pool.tile([128, NKC, DK], bf16, tag="v1")
                st_.update(QT=QT, KT=KT, v0=v0, v1=v1)
                units = []
                for st in range(NST):
                    for w_t, dst, is_q in ((wq_t, QT, True), (wk_t, KT, False)):
                        carry = {}

                        def u_p1(st=st, w_t=w_t, carry=carry):
                            sl = slice(st * 512, (st + 1) * 512)
                            pq = ps_pj.tile([128, 512], fp32, tag="pj")
                            carry["pq"] = pq
                            for cch in range(4):
                                nc.tensor.matmul(
                                    pq[:], w_t[:, cch, :], xT[:, cch, sl],
                                    start=(cch == 0), stop=False,
                                    skip_group_check=True,
                                )

                        def u_p2(st=st, w_t=w_t, dst=dst, is_q=is_q,
                                 carry=carry):
                            sl = slice(st * 512, (st + 1) * 512)
                            pq = carry["pq"]
                            for cch in range(4, NDC):
                                nc.tensor.matmul(
                                    pq[:], w_t[:, cch, :], xT[:, cch, sl],
                                    start=False, stop=(cch == NDC - 1),
                                    skip_group_check=True,
                                )
                            if is_q:
                                nc.vector.tensor_scalar_add(
                                    dst[:, sl], pq[:], bq_t[:]
                                )
                            else:
                                nc.vector.tensor_copy(dst[:, sl], pq[:])
                        units.append(u_p1)
                        units.append(u_p2)

                for g in range(4):  # groups of 4 s-blocks
                    vcarry = {}
                    for j in range(4):
                        def u_v(g=g, j=j, vcarry=vcarry):
                            if j == 0:
                                pv = ps_pj.tile([128, 512], fp32, tag="pj")
                                vcarry["pv"] = pv
                            pv = vcarry["pv"]
                            sb = g * 4 + j
                            qsl = slice(j * 128, (j + 1) * 128)
                            for cch in range(NDC):
                                nc.tensor.matmul(
                                    pv[:, qsl],
                                    xT[:, cch, sb * 128 : (sb + 1) * 128],
                                    wv_t[:, cch, :],
                                    start=(cch == 0), stop=(cch == NDC - 1),
                                    skip_group_check=True,
                                )
                            if j == 3:
                                pvv = pv[:].rearrange("p (j c) -> p j c", j=4)
                                nc.vector.tensor_copy(
                                    v0[:, g * 4 : (g + 1) * 4, :],
                                    pvv[:, :, 0:DK],
                                )
                                nc.vector.tensor_copy(
                                    v1[:, g * 4 : (g + 1) * 4, :],
                                    pvv[:, :, DK:CS],
                                )
                        units.append(u_v)
                return units

            def B_units(bi):
                """Attention for batch bi: 2 sections (one per q-pair).
                Both 512-subs of the q-pair are processed per k-chunk so
                each stationary (K_h, V_h) serves two matmuls and the 4
                sumexp matmuls run as one 4-way col-tiled pass."""
                st_ = state[bi]
                QT, KT, v0, v1 = st_["QT"], st_["KT"], st_["v0"], st_["v1"]
                ctxs = []
                sections = []
                for qp in range(NQP):
                    q0 = qp * 1024
                    ctx = ctx_pool.tile([128, 1024], bf16, tag="ctx")
                    ctxs.append(ctx)
                    carry = {}
                    units = []

                    def u_start(carry=carry):
                        av0 = ps_av.tile([128, 512], fp32, tag="av")
                        av1 = ps_av.tile([128, 512], fp32, tag="av")
                        se = ps_se.tile([128, 512], fp32, tag="se")
                        nc.vector.memset(se[:], 1.0)
                        carry.update(av=(av0, av1), se=se, ex={})
                    units.append(u_start)

                    def u_sc(kc, q0=q0, carry=carry):
                        ksl = slice(kc * 128, (kc + 1) * 128)
                        exs = []
                        for sub in range(2):
                            qsl = slice(q0 + sub * 512, q0 + (sub + 1) * 512)
                            sc = ps_sc.tile([128, 2, 512], fp32, tag="sc")
                            # h0/h1 adjacent -> concurrent row tiles (0/64);
                            # K stationaries persist across the two subs
                            nc.tensor.matmul(
                                sc[:, 0, :], KT[0:DK, ksl], QT[0:DK, qsl],
                                start=True, stop=True,
                            )
                            nc.tensor.matmul(
                                sc[:, 1, :], KT[DK:CS, ksl], QT[DK:CS, qsl],
                                start=True, stop=True,
                            )
                            ex = ex_pool.tile([128, 2, 512], bf16, tag="ex")
                            nc.scalar.activation(
                                ex[:], sc[:], AF.Exp, scale=0.125
                            )
                            exs.append(ex)
                        carry["ex"][kc] = exs

                    def u_av(kc, carry=carry):
                        ex0, ex1 = carry["ex"].pop(kc)
                        av0, av1 = carry["av"]
                        se = carry["se"]
                        first, last = kc == 0, kc == NKC - 1
                        # col-tiled pairs: h0 -> rows 0-63, h1 -> 64-127;
                        # V stationaries persist across the two subs
                        for ex, av in ((ex0, av0), (ex1, av1)):
                            nc.tensor.matmul(
                                av[0:DK, :], v0[:, kc, :], ex[:, 0, :],
                                start=first, stop=last, skip_group_check=True,
                            )
                            nc.tensor.matmul(
                                av[DK:CS, :], v1[:, kc, :], ex[:, 1, :],
                                start=first, stop=last, skip_group_check=True,
                            )
                        # sumexp: 4-way col-tiled pass, rows (sub, h) ->
                        # 0:(s0,h0) 32:(s0,h1) 64:(s1,h0) 96:(s1,h1)
                        for qi, ex in enumerate((ex0, ex0, ex1, ex1)):
                            p = 32 * qi
                            nc.tensor.matmul(
                                se[p : p + 1, :], ones_t[:], ex[:, qi % 2, :],
                                start=first, stop=last,
                                skip_group_check=True,
                                tile_position=(0, p),
                            )

                    for kc in range(NKC):
                        def u_kc(kc=kc, u_sc=u_sc, u_av=u_av):
                            u_sc(kc)
                            if kc > 0:
                                u_av(kc - 1)
                            if kc == NKC - 1:
                                u_av(kc)
                        units.append(u_kc)

                    def u_end(carry=carry, ctx=ctx):
                        av = carry["av"]
                        se = carry["se"]
                        avss = []
                        for s in range(2):
                            avs = avs_pool.tile([128, 512], fp32, tag="avs")
                            nc.vector.tensor_copy(avs[:], av[s][:])
                            avss.append(avs)
                        rse = rec_pool.tile([128, 512], fp32, tag="rse")
                        nc.vector.reciprocal(rse[:], se[:])
                        for s in range(2):
                            # rb rows 0-63 <- 1/se(h0), 64-127 <- 1/se(h1)
                            rb = rb_pool.tile([128, 512], fp32, tag="rb")
                            for h in range(2):
                                dr = dram_pool.tile([1, 512], fp32, tag="dr")
                                nc.sync.dma_start(
                                    dr[:],
                                    rse[64 * s + 32 * h : 64 * s + 32 * h + 1, :],
                                )
                                nc.sync.dma_start(
                                    rb[h * DK : (h + 1) * DK, :],
                                    dr[:].partition_broadcast(DK),
                                )
                            ssl = slice(s * 512, (s + 1) * 512)
                            nc.vector.tensor_mul(
                                ctx[:, ssl], avss[s][:], rb[:]
                            )
                    units.append(u_end)
                    sections.append(units)
                st_["ctx"] = ctxs
                return sections

            def C_units(bi, b, qp):
                ctx = state[bi]["ctx"][qp]
                units = []
                for j in range(8):
                    ocarry = {}
                    for half in range(2):
                        def u_o(j=j, half=half, ocarry=ocarry):
                            sb = qp * 8 + j
                            lsl = slice(j * 128, (j + 1) * 128)
                            if half == 0:
                                ot = out_pool.tile([128, D], bf16, tag="ot")
                                ocarry["ot"] = ot
                            ot = ocarry["ot"]
                            osl = slice(half * 512, (half + 1) * 512)
                            po = ps_pj.tile([128, 512], fp32, tag="pj")
                            nc.tensor.matmul(
                                po[:], ctx[:, lsl], wo_t[:, osl],
                                start=True, stop=True,
                            )
                            nc.vector.tensor_copy(ot[:, osl], po[:])
                            if half == 1:
                                nc.sync.dma_start(
                                    out_d[b, sb * 128 : (sb + 1) * 128, :],
                                    ot[:],
                                )
                        units.append(u_o)
                return units

            # ---- software pipeline over batches ----
            for u in A_xdma(0, 0):
                u()
            load_consts()
            a_cur = A_units(0)
            c_prev = []  # C units of (bi-1, qp1)
            for bi in range(B):
                for u in a_cur:
                    u()
                secs = B_units(bi)
                if bi + 1 < B:
                    for u in A_xdma(bi + 1, bi + 1):
                        u()
                    a_next = A_units(bi + 1)
                else:
                    a_next = []
                half = len(a_next) // 2
                # qp0 attention: filled with prev batch's qp1 out-proj and
                # the first half of next batch's projections
                for u in _interleave(secs[0], c_prev + a_next[:half], 0.12):
                    u()
                c_q0 = C_units(bi, bi, 0)
                for u in _interleave(secs[1], c_q0 + a_next[half:], 0.12):
                    u()
                c_prev = C_units(bi, bi, 1)
                a_cur = []
            for u in c_prev:
                u()

    nc.compile()
    return nc


def _get_nc():
    if "nc" not in _CACHE:
        _CACHE["nc"] = _build()
    return _CACHE["nc"]


def kernel(**inputs):
    global LAST_RESULTS
    import ml_dtypes
    from concourse.bass_utils import run_bass_kernel_spmd

    bf = ml_dtypes.bfloat16
    x = np.asarray(inputs["x"], dtype=np.float32)
    xt = np.ascontiguousarray(x.transpose(0, 2, 1)).astype(bf)  # [B, D, S]
    Wq = np.asarray(inputs["Wq"], dtype=np.float32).astype(bf)
    Wk = np.asarray(inputs["Wk"], dtype=np.float32).astype(bf)
    Wv = np.asarray(inputs["Wv"], dtype=np.float32).astype(bf)
    Wo = np.asarray(inputs["Wo"], dtype=np.float32).astype(bf)
    bq = np.asarray(inputs["bq"], dtype=np.float32)
    bv = np.asarray(inputs["bv"], dtype=np.float32)
    bo = np.asarray(inputs["bo"], dtype=np.float32)

    nc = _get_nc()
    in_maps = []
    for c in range(NCORES):
        cs = slice(CS * c, CS * (c + 1))
        in_maps.append(
            {
                "xt": xt,
                "wq": np.ascontiguousarray(Wq[:, cs]),
                "wk": np.ascontiguousarray(Wk[:, cs]),
                "wv": np.ascontiguousarray(Wv[:, cs]),
                "wo": np.ascontiguousarray(Wo[cs, :]),
                "bq": np.ascontiguousarray(bq[cs]),
            }
        )
    res = run_bass_kernel_spmd(
        nc, in_maps, core_ids=list(range(NCORES)), trace=TRACE
    )
    LAST_RESULTS = res
    acc = np.zeros((B, S, D), dtype=np.float64)
    for c in range(NCORES):
        acc += np.asarray(res.results[c]["out"], dtype=np.float64)
    # bk drops out of softmax; bv commutes through (sum of weights = 1)
    acc += bo + bv.astype(np.float64) @ np.asarray(
        inputs["Wo"], dtype=np.float64
    )
    return acc.astype(np.float32)


# revision 17
# speedup vs baseline: 1.2781x; 1.0261x over previous
"""Multi-head attention on 8 Trainium2 NeuronCores (tensor-parallel over heads).

B=4, S=2048, D=1024, H=16 heads of DK=64. Each core owns 2 heads (a
128-channel slice of the QKV projections). x is pre-transposed on the
host to [B, D, S] so the device reads contiguous rows (no DMA transpose).

Per core, per batch b (all matmul operands bf16, fp32 PSUM accumulate):
  QT   = Wq_c^T xT + bq    [128, S]   (both heads stacked on partitions)
  KT   = Wk_c^T xT         [128, S]   (bk cancels in softmax -- dropped)
  V    = xT^T Wv_c         [S, 128] stored per-head [128, 16, 64]
  attention per (q-pair, 512-wide sub), accumulating over k-chunks kc:
    sc[128, 2, 512] psum = K_h Q_h^T for h0, h1 -- the two C=64 matmuls
       are issued adjacently so they run concurrently in the PE array
       (row tiling at partitions 0/64); sc double-buffered so ACT and PE
       ping-pong without stalls
    ex = exp(sc/8)  one ACT instr, N=1024 (amortizes the 352-cyc overhead)
    av[128, 512] psum += [V0^T ex0 ; V1^T ex1]  (col-tiled pair, M=64
       each at col positions 0/64 -- ctx^T lands in O-projection layout)
    se psum += ones^T ex   (2 col-tiled M=1 matmuls; rows 0/32 for sub0,
       64/96 for sub1, one se bank per q-pair)
  rse = 1/se (one DVE reciprocal per q-pair), broadcast via DRAM bounce,
  ctx = av * rse on DVE, out partial = ctx^T Wo_c  [S, D] bf16.
Host sums the 8 cores' partials and adds bo + bv@Wo (bv commutes through
softmax since the attention weights sum to 1).
"""

import numpy as np

B, S, D, H, DK = 4, 2048, 1024, 16, 64
NCORES = 8
CS = D // NCORES   # 128 channels (2 heads) per core
NDC = D // 128     # 8 d-chunks
NKC = S // 128     # 16 k-chunks
NST = S // 512     # 4 s-tiles
NQP = S // 1024    # 2 q-pairs

TRACE = False
LAST_RESULTS = None
_CACHE = {}


def _interleave(main, fill, start_frac=0.2):
    """Spread fill units evenly between main units (order preserved).
    No fill before start_frac of main has been emitted: the engines run
    in static order, so a fill unit whose inputs aren't ready yet would
    stall them."""
    out = []
    fi = 0
    n0 = int(len(main) * start_frac)
    for i, u in enumerate(main):
        out.append(u)
        if i < n0:
            continue
        want = (i - n0 + 1) * len(fill) // max(1, len(main) - n0)
        while fi < want:
            out.append(fill[fi])
            fi += 1
    out.extend(fill[fi:])
    return out


def _build():
    import concourse.bass as bass  # noqa: F401
    import concourse.mybir as mybir
    import concourse.tile as tile
    from concourse import bacc

    fp32 = mybir.dt.float32
    bf16 = mybir.dt.bfloat16
    AF = mybir.ActivationFunctionType

    nc = bacc.Bacc(None, target_bir_lowering=False)
    xt_d = nc.declare_dram_parameter("xt", [B, D, S], bf16, isOutput=False)
    out_d = nc.declare_dram_parameter("out", [B, S, D], bf16, isOutput=True)
    wq_d = nc.declare_dram_parameter("wq", [128, NDC, CS], bf16, isOutput=False)
    wk_d = nc.declare_dram_parameter("wk", [128, NDC, CS], bf16, isOutput=False)
    wv_d = nc.declare_dram_parameter("wv", [128, NDC, CS], bf16, isOutput=False)
    wo_d = nc.declare_dram_parameter("wo", [CS, D], bf16, isOutput=False)
    bq_d = nc.declare_dram_parameter("bq", [CS], fp32, isOutput=False)

    with tile.TileContext(nc) as tc:
        with (
            tc.tile_pool(name="consts", bufs=1) as consts,
            tc.tile_pool(name="xt", bufs=2) as xt_pool,
            tc.tile_pool(name="qk", bufs=2) as qk_pool,
            tc.tile_pool(name="vp", bufs=2) as v_pool,
            tc.tile_pool(name="exq", bufs=6) as ex_pool,
            tc.tile_pool(name="ctx", bufs=2) as ctx_pool,
            tc.tile_pool(name="avs", bufs=4) as avs_pool,
            tc.tile_pool(name="rec", bufs=2) as rec_pool,
            tc.tile_pool(name="rb", bufs=8) as rb_pool,
            tc.tile_pool(name="outp", bufs=4) as out_pool,
            tc.tile_pool(name="drp", bufs=8, space="DRAM") as dram_pool,
            tc.tile_pool(name="pssc", bufs=2, space="PSUM") as ps_sc,
            tc.tile_pool(name="psav", bufs=2, space="PSUM") as ps_av,
            tc.tile_pool(name="psse", bufs=1, space="PSUM") as ps_se,
            tc.tile_pool(name="pspj", bufs=1, space="PSUM") as ps_pj,
        ):
            wq_t = consts.tile([128, NDC, CS], bf16, tag="wq")
            wk_t = consts.tile([128, NDC, CS], bf16, tag="wk")
            wv_t = consts.tile([128, NDC, CS], bf16, tag="wv")
            wo_t = consts.tile([128, D], bf16, tag="wo")
            bq_t = consts.tile([128, 1], fp32, tag="bq")
            ones_t = consts.tile([128, 1], bf16, tag="ones")

            def load_consts():
                # wq/wk/wv arrive host-permuted as [128, NDC, CS] so these
                # are contiguous row DMAs
                nc.sync.dma_start(wq_t[:], wq_d[:])
                nc.sync.dma_start(wk_t[:], wk_d[:])
                nc.sync.dma_start(wv_t[:], wv_d[:])
                nc.sync.dma_start(wo_t[:], wo_d[:])
                nc.sync.dma_start(bq_t[:], bq_d[:].rearrange("(p o) -> p o", o=1))
                nc.gpsimd.memset(ones_t[:], 1.0)

            state = {}

            def A_xdma(bi, b):
                xT = xt_pool.tile([128, NDC, S], bf16, tag="xT")
                state[bi] = dict(xT=xT)
                xr = xt_d[b].rearrange("(c p) M -> p c M", p=128)
                return [
                    (lambda cch=cch: nc.sync.dma_start(
                        xT[:, cch, :], xr[:, cch]))
                    for cch in range(NDC)
                ]

            def A_units(bi):
                st_ = state[bi]
                xT = st_["xT"]
                QT = qk_pool.tile([128, S], bf16, tag="QT")
                KT = qk_pool.tile([128, S], bf16, tag="KT")
                v0 = v_pool.tile([128, NKC, DK], bf16, tag="v0")
                v1 = v_pool.tile([128, NKC, DK], bf16, tag="v1")
                st_.update(QT=QT, KT=KT, v0=v0, v1=v1)
                units = []
                for st in range(NST):
                    for w_t, dst, is_q in ((wq_t, QT, True), (wk_t, KT, False)):
                        carry = {}

                        def u_p1(st=st, w_t=w_t, carry=carry):
                            sl = slice(st * 512, (st + 1) * 512)
                            pq = ps_pj.tile([128, 512], fp32, tag="pj")
                            carry["pq"] = pq
                            for cch in range(4):
                                nc.tensor.matmul(
                                    pq[:], w_t[:, cch, :], xT[:, cch, sl],
                                    start=(cch == 0), stop=False,
                                    skip_group_check=True,
                                )

                        def u_p2(st=st, w_t=w_t, dst=dst, is_q=is_q,
                                 carry=carry):
                            sl = slice(st * 512, (st + 1) * 512)
                            pq = carry["pq"]
                            for cch in range(4, NDC):
                                nc.tensor.matmul(
                                    pq[:], w_t[:, cch, :], xT[:, cch, sl],
                                    start=False, stop=(cch == NDC - 1),
                                    skip_group_check=True,
                                )
                            if is_q:
                                nc.vector.tensor_scalar_add(
                                    dst[:, sl], pq[:], bq_t[:]
                                )
                            else:
                                nc.vector.tensor_copy(dst[:, sl], pq[:])
                        units.append(u_p1)
                        units.append(u_p2)

                for g in range(4):  # groups of 4 s-blocks
                    vcarry = {}
                    for j in range(4):
                        def u_v(g=g, j=j, vcarry=vcarry):
                            if j == 0:
                                pv = ps_pj.tile([128, 512], fp32, tag="pj")
                                vcarry["pv"] = pv
                            pv = vcarry["pv"]
                            sb = g * 4 + j
                            qsl = slice(j * 128, (j + 1) * 128)
                            for cch in range(NDC):
                                nc.tensor.matmul(
                                    pv[:, qsl],
                                    xT[:, cch, sb * 128 : (sb + 1) * 128],
                                    wv_t[:, cch, :],
                                    start=(cch == 0), stop=(cch == NDC - 1),
                                    skip_group_check=True,
                                )
                            if j == 3:
                                pvv = pv[:].rearrange("p (j c) -> p j c", j=4)
                                nc.vector.tensor_copy(
                                    v0[:, g * 4 : (g + 1) * 4, :],
                                    pvv[:, :, 0:DK],
                                )
                                nc.vector.tensor_copy(
                                    v1[:, g * 4 : (g + 1) * 4, :],
                                    pvv[:, :, DK:CS],
                                )
                        units.append(u_v)
                return units

            def B_units(bi):
                """Attention for batch bi: 2 sections (one per q-pair).
                Both 512-subs of the q-pair are processed per k-chunk so
                each stationary (K_h, V_h) serves two matmuls and the 4
                sumexp matmuls run as one 4-way col-tiled pass."""
                st_ = state[bi]
                QT, KT, v0, v1 = st_["QT"], st_["KT"], st_["v0"], st_["v1"]
                ctxs = []
                sections = []
                for qp in range(NQP):
                    q0 = qp * 1024
                    ctx = ctx_pool.tile([128, 1024], bf16, tag="ctx")
                    ctxs.append(ctx)
                    carry = {}
                    units = []

                    def u_start(carry=carry):
                        av0 = ps_av.tile([128, 512], fp32, tag="av")
                        av1 = ps_av.tile([128, 512], fp32, tag="av")
                        se = ps_se.tile([128, 512], fp32, tag="se")
                        nc.vector.memset(se[:], 1.0)
                        carry.update(av=(av0, av1), se=se, ex={})
                    units.append(u_start)

                    def u_sc(kc, q0=q0, carry=carry):
                        ksl = slice(kc * 128, (kc + 1) * 128)
                        exs = []
                        for sub in range(2):
                            qsl = slice(q0 + sub * 512, q0 + (sub + 1) * 512)
                            sc = ps_sc.tile([128, 2, 512], fp32, tag="sc")
                            # h0/h1 adjacent -> concurrent row tiles (0/64);
                            # K stationaries persist across the two subs
                            nc.tensor.matmul(
                                sc[:, 0, :], KT[0:DK, ksl], QT[0:DK, qsl],
                                start=True, stop=True,
                            )
                            nc.tensor.matmul(
                                sc[:, 1, :], KT[DK:CS, ksl], QT[DK:CS, qsl],
                                start=True, stop=True,
                            )
                            ex = ex_pool.tile([128, 2, 512], bf16, tag="ex")
                            nc.scalar.activation(
                                ex[:], sc[:], AF.Exp, scale=0.125
                            )
                            exs.append(ex)
                        carry["ex"][kc] = exs

                    def u_av(kc, carry=carry):
                        ex0, ex1 = carry["ex"].pop(kc)
                        av0, av1 = carry["av"]
                        se = carry["se"]
                        first, last = kc == 0, kc == NKC - 1
                        # col-tiled pairs: h0 -> rows 0-63, h1 -> 64-127;
                        # V stationaries persist across the two subs
                        for ex, av in ((ex0, av0), (ex1, av1)):
                            nc.tensor.matmul(
                                av[0:DK, :], v0[:, kc, :], ex[:, 0, :],
                                start=first, stop=last, skip_group_check=True,
                            )
                            nc.tensor.matmul(
                                av[DK:CS, :], v1[:, kc, :], ex[:, 1, :],
                                start=first, stop=last, skip_group_check=True,
                            )
                        # sumexp: 4-way col-tiled pass, rows (sub, h) ->
                        # 0:(s0,h0) 32:(s0,h1) 64:(s1,h0) 96:(s1,h1)
                        for qi, ex in enumerate((ex0, ex0, ex1, ex1)):
                            p = 32 * qi
                            nc.tensor.matmul(
                                se[p : p + 1, :], ones_t[:], ex[:, qi % 2, :],
                                start=first, stop=last,
                                skip_group_check=True,
                                tile_position=(0, p),
                            )

                    for kc in range(NKC):
                        def u_kc(kc=kc, u_sc=u_sc, u_av=u_av):
                            u_sc(kc)
                            if kc > 0:
                                u_av(kc - 1)
                            if kc == NKC - 1:
                                u_av(kc)
                        units.append(u_kc)

                    def u_end(carry=carry, ctx=ctx):
                        av = carry["av"]
                        se = carry["se"]
                        avss = []
                        for s in range(2):
                            avs = avs_pool.tile([128, 512], fp32, tag="avs")
                            nc.vector.tensor_copy(avs[:], av[s][:])
                            avss.append(avs)
                        rse = rec_pool.tile([128, 512], fp32, tag="rse")
                        nc.vector.reciprocal(rse[:], se[:])
                        for s in range(2):
                            # rb rows 0-63 <- 1/se(h0), 64-127 <- 1/se(h1)
                            rb = rb_pool.tile([128, 512], fp32, tag="rb")
                            for h in range(2):
                                dr = dram_pool.tile([1, 512], fp32, tag="dr")
                                nc.sync.dma_start(
                                    dr[:],
                                    rse[64 * s + 32 * h : 64 * s + 32 * h + 1, :],
                                )
                                nc.sync.dma_start(
                                    rb[h * DK : (h + 1) * DK, :],
                                    dr[:].partition_broadcast(DK),
                                )
                            ssl = slice(s * 512, (s + 1) * 512)
                            nc.vector.tensor_mul(
                                ctx[:, ssl], avss[s][:], rb[:]
                            )
                    units.append(u_end)
                    sections.append(units)
                st_["ctx"] = ctxs
                return sections

            def C_units(bi, b, qp):
                ctx = state[bi]["ctx"][qp]
                units = []
                for j in range(8):
                    ocarry = {}
                    for half in range(2):
                        def u_o(j=j, half=half, ocarry=ocarry):
                            sb = qp * 8 + j
                            lsl = slice(j * 128, (j + 1) * 128)
                            if half == 0:
                                ot = out_pool.tile([128, D], bf16, tag="ot")
                                ocarry["ot"] = ot
                            ot = ocarry["ot"]
                            osl = slice(half * 512, (half + 1) * 512)
                            po = ps_pj.tile([128, 512], fp32, tag="pj")
                            nc.tensor.matmul(
                                po[:], ctx[:, lsl], wo_t[:, osl],
                                start=True, stop=True,
                            )
                            nc.vector.tensor_copy(ot[:, osl], po[:])
                            if half == 1:
                                nc.sync.dma_start(
                                    out_d[b, sb * 128 : (sb + 1) * 128, :],
                                    ot[:],
                                )
                        units.append(u_o)
                return units

            # ---- software pipeline over batches ----
            for u in A_xdma(0, 0):
                u()
            load_consts()
            a_cur = A_units(0)
            c_prev = []  # C units of (bi-1, qp1)
            for bi in range(B):
                for u in a_cur:
                    u()
                secs = B_units(bi)
                if bi + 1 < B:
                    for u in A_xdma(bi + 1, bi + 1):
                        u()
                    a_next = A_units(bi + 1)
                else:
                    a_next = []
                half = len(a_next) // 2
                # qp0 attention: filled with prev batch's qp1 out-proj and
                # the first half of next batch's projections. The C units
                # go AFTER the projection fills so the normalize DRAM
                # bounce they depend on has time to land.
                for u in _interleave(secs[0], a_next[:half] + c_prev, 0.12):
                    u()
                c_q0 = C_units(bi, bi, 0)
                for u in _interleave(secs[1], a_next[half:] + c_q0, 0.12):
                    u()
                c_prev = C_units(bi, bi, 1)
                a_cur = []
            for u in c_prev:
                u()

    nc.compile()
    return nc


def _get_nc():
    if "nc" not in _CACHE:
        _CACHE["nc"] = _build()
    return _CACHE["nc"]


def kernel(**inputs):
    global LAST_RESULTS
    import ml_dtypes
    from concourse.bass_utils import run_bass_kernel_spmd

    bf = ml_dtypes.bfloat16
    x = np.asarray(inputs["x"], dtype=np.float32)
    xt = np.ascontiguousarray(x.transpose(0, 2, 1)).astype(bf)  # [B, D, S]
    Wq = np.asarray(inputs["Wq"], dtype=np.float32).astype(bf)
    Wk = np.asarray(inputs["Wk"], dtype=np.float32).astype(bf)
    Wv = np.asarray(inputs["Wv"], dtype=np.float32).astype(bf)
    Wo = np.asarray(inputs["Wo"], dtype=np.float32).astype(bf)
    bq = np.asarray(inputs["bq"], dtype=np.float32)
    bv = np.asarray(inputs["bv"], dtype=np.float32)
    bo = np.asarray(inputs["bo"], dtype=np.float32)

    def permute_w(w):  # [D, CS] -> [128, NDC, CS] (partition-major chunks)
        return np.ascontiguousarray(
            w.reshape(NDC, 128, CS).transpose(1, 0, 2)
        )

    nc = _get_nc()
    in_maps = []
    for c in range(NCORES):
        cs = slice(CS * c, CS * (c + 1))
        in_maps.append(
            {
                "xt": xt,
                "wq": permute_w(Wq[:, cs]),
                "wk": permute_w(Wk[:, cs]),
                "wv": permute_w(Wv[:, cs]),
                "wo": np.ascontiguousarray(Wo[cs, :]),
                "bq": np.ascontiguousarray(bq[cs]),
            }
        )
    res = run_bass_kernel_spmd(
        nc, in_maps, core_ids=list(range(NCORES)), trace=TRACE
    )
    LAST_RESULTS = res
    acc = np.zeros((B, S, D), dtype=np.float64)
    for c in range(NCORES):
        acc += np.asarray(res.results[c]["out"], dtype=np.float64)
    # bk drops out of softmax; bv commutes through (sum of weights = 1)
    acc += bo + bv.astype(np.float64) @ np.asarray(
        inputs["Wo"], dtype=np.float64
    )
    return acc.astype(np.float32)


# revision 21
# speedup vs baseline: 1.2821x; 1.0032x over previous
"""Multi-head attention on 8 Trainium2 NeuronCores (tensor-parallel over heads).

B=4, S=2048, D=1024, H=16 heads of DK=64. Each core owns 2 heads (a
128-channel slice of the QKV projections). x is pre-transposed on the
host to [B, D, S] so the device reads contiguous rows (no DMA transpose).

Per core, per batch b (all matmul operands bf16, fp32 PSUM accumulate):
  QT   = Wq_c^T xT + bq    [128, S]   (both heads stacked on partitions)
  KT   = Wk_c^T xT         [128, S]   (bk cancels in softmax -- dropped)
  V    = xT^T Wv_c         [S, 128] stored per-head [128, 16, 64]
  attention per (q-pair, 512-wide sub), accumulating over k-chunks kc:
    sc[128, 2, 512] psum = K_h Q_h^T for h0, h1 -- the two C=64 matmuls
       are issued adjacently so they run concurrently in the PE array
       (row tiling at partitions 0/64); sc double-buffered so ACT and PE
       ping-pong without stalls
    ex = exp(sc/8)  one ACT instr, N=1024 (amortizes the 352-cyc overhead)
    av[128, 512] psum += [V0^T ex0 ; V1^T ex1]  (col-tiled pair, M=64
       each at col positions 0/64 -- ctx^T lands in O-projection layout)
    se psum += ones^T ex   (2 col-tiled M=1 matmuls; rows 0/32 for sub0,
       64/96 for sub1, one se bank per q-pair)
  rse = 1/se (one DVE reciprocal per q-pair), broadcast via DRAM bounce,
  ctx = av * rse on DVE, out partial = ctx^T Wo_c  [S, D] bf16.
Host sums the 8 cores' partials and adds bo + bv@Wo (bv commutes through
softmax since the attention weights sum to 1).
"""

import numpy as np

B, S, D, H, DK = 4, 2048, 1024, 16, 64
NCORES = 8
CS = D // NCORES   # 128 channels (2 heads) per core
NDC = D // 128     # 8 d-chunks
NKC = S // 128     # 16 k-chunks
NST = S // 512     # 4 s-tiles
NQP = S // 1024    # 2 q-pairs

TRACE = False
LAST_RESULTS = None
_CACHE = {}


def _interleave(main, fill, start_frac=0.2):
    """Spread fill units evenly between main units (order preserved).
    No fill before start_frac of main has been emitted: the engines run
    in static order, so a fill unit whose inputs aren't ready yet would
    stall them."""
    out = []
    fi = 0
    n0 = int(len(main) * start_frac)
    for i, u in enumerate(main):
        out.append(u)
        if i < n0:
            continue
        want = (i - n0 + 1) * len(fill) // max(1, len(main) - n0)
        while fi < want:
            out.append(fill[fi])
            fi += 1
    out.extend(fill[fi:])
    return out


def _build():
    import concourse.bass as bass  # noqa: F401
    import concourse.mybir as mybir
    import concourse.tile as tile
    from concourse import bacc

    fp32 = mybir.dt.float32
    bf16 = mybir.dt.bfloat16
    AF = mybir.ActivationFunctionType

    nc = bacc.Bacc(None, target_bir_lowering=False)
    xt_d = nc.declare_dram_parameter("xt", [B, D, S], bf16, isOutput=False)
    out_d = nc.declare_dram_parameter("out", [B, S, D], bf16, isOutput=True)
    wq_d = nc.declare_dram_parameter("wq", [128, NDC, CS], bf16, isOutput=False)
    wk_d = nc.declare_dram_parameter("wk", [128, NDC, CS], bf16, isOutput=False)
    wv_d = nc.declare_dram_parameter("wv", [128, NDC, CS], bf16, isOutput=False)
    wo_d = nc.declare_dram_parameter("wo", [CS, D], bf16, isOutput=False)
    bq_d = nc.declare_dram_parameter("bq", [CS], fp32, isOutput=False)

    with tile.TileContext(nc) as tc:
        with (
            tc.tile_pool(name="consts", bufs=1) as consts,
            tc.tile_pool(name="xt", bufs=2) as xt_pool,
            tc.tile_pool(name="qk", bufs=2) as qk_pool,
            tc.tile_pool(name="vp", bufs=2) as v_pool,
            tc.tile_pool(name="exq", bufs=6) as ex_pool,
            tc.tile_pool(name="ctx", bufs=2) as ctx_pool,
            tc.tile_pool(name="avs", bufs=4) as avs_pool,
            tc.tile_pool(name="rec", bufs=2) as rec_pool,
            tc.tile_pool(name="rb", bufs=8) as rb_pool,
            tc.tile_pool(name="outp", bufs=4) as out_pool,
            tc.tile_pool(name="drp", bufs=8, space="DRAM") as dram_pool,
            tc.tile_pool(name="pssc", bufs=2, space="PSUM") as ps_sc,
            tc.tile_pool(name="psav", bufs=2, space="PSUM") as ps_av,
            tc.tile_pool(name="psse", bufs=1, space="PSUM") as ps_se,
            tc.tile_pool(name="pspj", bufs=1, space="PSUM") as ps_pj,
        ):
            wq_t = consts.tile([128, NDC, CS], bf16, tag="wq")
            wk_t = consts.tile([128, NDC, CS], bf16, tag="wk")
            wv_t = consts.tile([128, NDC, CS], bf16, tag="wv")
            wo_t = consts.tile([128, D], bf16, tag="wo")
            bq_t = consts.tile([128, 1], fp32, tag="bq")
            ones_t = consts.tile([128, 1], bf16, tag="ones")

            def load_consts():
                # wq/wk/wv arrive host-permuted as [128, NDC, CS] so these
                # are contiguous row DMAs
                nc.sync.dma_start(wq_t[:], wq_d[:])
                nc.sync.dma_start(wk_t[:], wk_d[:])
                nc.sync.dma_start(wv_t[:], wv_d[:])
                nc.sync.dma_start(wo_t[:], wo_d[:])
                nc.sync.dma_start(bq_t[:], bq_d[:].rearrange("(p o) -> p o", o=1))
                nc.gpsimd.memset(ones_t[:], 1.0)

            state = {}

            def A_xdma(bi, b):
                xT = xt_pool.tile([128, NDC, S], bf16, tag="xT")
                state[bi] = dict(xT=xT)
                xr = xt_d[b].rearrange("(c p) M -> p c M", p=128)
                return [
                    (lambda cch=cch: nc.sync.dma_start(
                        xT[:, cch, :], xr[:, cch]))
                    for cch in range(NDC)
                ]

            def A_units(bi):
                st_ = state[bi]
                xT = st_["xT"]
                QT = qk_pool.tile([128, S], bf16, tag="QT")
                KT = qk_pool.tile([128, S], bf16, tag="KT")
                v0 = v_pool.tile([128, NKC, DK], bf16, tag="v0")
                v1 = v_pool.tile([128, NKC, DK], bf16, tag="v1")
                st_.update(QT=QT, KT=KT, v0=v0, v1=v1)
                # All projections run as F=128 accumulation chains packing
                # four [128,128] quarters into one psum bank -- this keeps
                # LDWEIGHTS pipelined (measured ~61ns/MM vs ~300ns at F=512)
                units = []
                for w_t, dst, is_q in ((wq_t, QT, True), (wk_t, KT, False)):
                    for g in range(4):
                        qcarry = {}
                        for j in range(4):
                            def u_p(g=g, j=j, w_t=w_t, dst=dst, is_q=is_q,
                                    qcarry=qcarry):
                                if j == 0:
                                    pq = ps_pj.tile([128, 512], fp32, tag="pj")
                                    qcarry["pq"] = pq
                                pq = qcarry["pq"]
                                sb = g * 4 + j
                                qsl = slice(j * 128, (j + 1) * 128)
                                for cch in range(NDC):
                                    nc.tensor.matmul(
                                        pq[:, qsl], w_t[:, cch, :],
                                        xT[:, cch, sb * 128 : (sb + 1) * 128],
                                        start=(cch == 0), stop=(cch == NDC - 1),
                                        skip_group_check=True,
                                    )
                                if j == 3:
                                    sl = slice(g * 512, (g + 1) * 512)
                                    if is_q:
                                        nc.vector.tensor_scalar_add(
                                            dst[:, sl], pq[:], bq_t[:]
                                        )
                                    else:
                                        nc.vector.tensor_copy(dst[:, sl], pq[:])
                            units.append(u_p)

                for g in range(4):  # groups of 4 s-blocks
                    vcarry = {}
                    for j in range(4):
                        def u_v(g=g, j=j, vcarry=vcarry):
                            if j == 0:
                                pv = ps_pj.tile([128, 512], fp32, tag="pj")
                                vcarry["pv"] = pv
                            pv = vcarry["pv"]
                            sb = g * 4 + j
                            qsl = slice(j * 128, (j + 1) * 128)
                            for cch in range(NDC):
                                nc.tensor.matmul(
                                    pv[:, qsl],
                                    xT[:, cch, sb * 128 : (sb + 1) * 128],
                                    wv_t[:, cch, :],
                                    start=(cch == 0), stop=(cch == NDC - 1),
                                    skip_group_check=True,
                                )
                            if j == 3:
                                pvv = pv[:].rearrange("p (j c) -> p j c", j=4)
                                nc.vector.tensor_copy(
                                    v0[:, g * 4 : (g + 1) * 4, :],
                                    pvv[:, :, 0:DK],
                                )
                                nc.vector.tensor_copy(
                                    v1[:, g * 4 : (g + 1) * 4, :],
                                    pvv[:, :, DK:CS],
                                )
                        units.append(u_v)
                return units

            def B_units(bi):
                """Attention for batch bi: 2 sections (one per q-pair).
                Both 512-subs of the q-pair are processed per k-chunk so
                each stationary (K_h, V_h) serves two matmuls and the 4
                sumexp matmuls run as one 4-way col-tiled pass."""
                st_ = state[bi]
                QT, KT, v0, v1 = st_["QT"], st_["KT"], st_["v0"], st_["v1"]
                ctxs = []
                sections = []
                for qp in range(NQP):
                    q0 = qp * 1024
                    ctx = ctx_pool.tile([128, 1024], bf16, tag="ctx")
                    ctxs.append(ctx)
                    carry = {}
                    units = []

                    def u_start(carry=carry):
                        av0 = ps_av.tile([128, 512], fp32, tag="av")
                        av1 = ps_av.tile([128, 512], fp32, tag="av")
                        se = ps_se.tile([128, 512], fp32, tag="se")
                        nc.vector.memset(se[:], 1.0)
                        carry.update(av=(av0, av1), se=se, ex={})
                    units.append(u_start)

                    def u_sc(kc, q0=q0, carry=carry):
                        ksl = slice(kc * 128, (kc + 1) * 128)
                        exs = []
                        for sub in range(2):
                            qsl = slice(q0 + sub * 512, q0 + (sub + 1) * 512)
                            sc = ps_sc.tile([128, 2, 512], fp32, tag="sc")
                            # h0/h1 adjacent -> concurrent row tiles (0/64);
                            # K stationaries persist across the two subs
                            nc.tensor.matmul(
                                sc[:, 0, :], KT[0:DK, ksl], QT[0:DK, qsl],
                                start=True, stop=True,
                            )
                            nc.tensor.matmul(
                                sc[:, 1, :], KT[DK:CS, ksl], QT[DK:CS, qsl],
                                start=True, stop=True,
                            )
                            ex = ex_pool.tile([128, 2, 512], bf16, tag="ex")
                            nc.scalar.activation(
                                ex[:], sc[:], AF.Exp, scale=0.125
                            )
                            exs.append(ex)
                        carry["ex"][kc] = exs

                    def u_av(kc, carry=carry):
                        ex0, ex1 = carry["ex"].pop(kc)
                        av0, av1 = carry["av"]
                        se = carry["se"]
                        first, last = kc == 0, kc == NKC - 1
                        # col-tiled pairs: h0 -> rows 0-63, h1 -> 64-127;
                        # V stationaries persist across the two subs
                        for ex, av in ((ex0, av0), (ex1, av1)):
                            nc.tensor.matmul(
                                av[0:DK, :], v0[:, kc, :], ex[:, 0, :],
                                start=first, stop=last, skip_group_check=True,
                            )
                            nc.tensor.matmul(
                                av[DK:CS, :], v1[:, kc, :], ex[:, 1, :],
                                start=first, stop=last, skip_group_check=True,
                            )
                        # sumexp: 4-way col-tiled pass, rows (sub, h) ->
                        # 0:(s0,h0) 32:(s0,h1) 64:(s1,h0) 96:(s1,h1)
                        for qi, ex in enumerate((ex0, ex0, ex1, ex1)):
                            p = 32 * qi
                            nc.tensor.matmul(
                                se[p : p + 1, :], ones_t[:], ex[:, qi % 2, :],
                                start=first, stop=last,
                                skip_group_check=True,
                                tile_position=(0, p),
                            )

                    for kc in range(NKC):
                        def u_kc(kc=kc, u_sc=u_sc, u_av=u_av):
                            u_sc(kc)
                            if kc > 0:
                                u_av(kc - 1)
                            if kc == NKC - 1:
                                u_av(kc)
                        units.append(u_kc)

                    def u_end(carry=carry):
                        # drain the av/se psum banks quickly; the rest of
                        # the normalize chain (u_norm) is emitted a few
                        # units into the next section so the 3 DVE ops +
                        # DRAM bounce don't block the pipeline here
                        av = carry["av"]
                        avss = []
                        for s in range(2):
                            avs = avs_pool.tile([128, 512], fp32, tag="avs")
                            nc.vector.tensor_copy(avs[:], av[s][:])
                            avss.append(avs)
                        rse = rec_pool.tile([128, 512], fp32, tag="rse")
                        nc.vector.reciprocal_approx_fast(rse[:], carry["se"][:])
                        carry.update(avss=avss, rse=rse)
                    units.append(u_end)

                    def u_norm(carry=carry, ctx=ctx):
                        rse = carry["rse"]
                        for s in range(2):
                            # rb rows 0-63 <- 1/se(h0), 64-127 <- 1/se(h1)
                            rb = rb_pool.tile([128, 512], fp32, tag="rb")
                            for h in range(2):
                                dr = dram_pool.tile([1, 512], fp32, tag="dr")
                                nc.sync.dma_start(
                                    dr[:],
                                    rse[64 * s + 32 * h : 64 * s + 32 * h + 1, :],
                                )
                                nc.sync.dma_start(
                                    rb[h * DK : (h + 1) * DK, :],
                                    dr[:].partition_broadcast(DK),
                                )
                            ssl = slice(s * 512, (s + 1) * 512)
                            nc.vector.tensor_mul(
                                ctx[:, ssl], carry["avss"][s][:], rb[:]
                            )
                    sections.append((units, u_norm))
                st_["ctx"] = ctxs
                return sections

            def C_units(bi, b, qp):
                ctx = state[bi]["ctx"][qp]
                units = []
                for j in range(8):
                    ocarry = {}
                    for half in range(2):
                        def u_o(j=j, half=half, ocarry=ocarry):
                            sb = qp * 8 + j
                            lsl = slice(j * 128, (j + 1) * 128)
                            if half == 0:
                                ot = out_pool.tile([128, D], bf16, tag="ot")
                                ocarry["ot"] = ot
                            ot = ocarry["ot"]
                            osl = slice(half * 512, (half + 1) * 512)
                            po = ps_pj.tile([128, 512], fp32, tag="pj")
                            for oc in range(4):
                                nc.tensor.matmul(
                                    po[:, oc * 128 : (oc + 1) * 128],
                                    ctx[:, lsl],
                                    wo_t[:, half * 512 + oc * 128 :
                                         half * 512 + (oc + 1) * 128],
                                    start=True, stop=True,
                                    skip_group_check=True,
                                )
                            nc.vector.tensor_copy(ot[:, osl], po[:])
                            if half == 1:
                                nc.sync.dma_start(
                                    out_d[b, sb * 128 : (sb + 1) * 128, :],
                                    ot[:],
                                )
                        units.append(u_o)
                return units

            # ---- software pipeline over batches ----
            for u in A_xdma(0, 0):
                u()
            load_consts()
            a_cur = A_units(0)
            c_prev = []   # C units of (bi-1, qp1)
            norm_prev = []  # deferred normalize of (bi-1, qp1)
            for bi in range(B):
                for u in a_cur:
                    u()
                (sec0, norm0), (sec1, norm1) = B_units(bi)
                if bi + 1 < B:
                    for u in A_xdma(bi + 1, bi + 1):
                        u()
                    a_next = A_units(bi + 1)
                else:
                    a_next = []
                half = len(a_next) // 2
                # Fill order: deferred normalize first (its C consumers sit
                # at the back of the list), then next batch's projections,
                # then the out-projection of the most recent ctx.
                for u in _interleave(
                    sec0, norm_prev + a_next[:half] + c_prev, 0.08
                ):
                    u()
                c_q0 = C_units(bi, bi, 0)
                for u in _interleave(
                    sec1, [norm0] + a_next[half:] + c_q0, 0.08
                ):
                    u()
                c_prev = C_units(bi, bi, 1)
                norm_prev = [norm1]
                a_cur = []
            for u in norm_prev:
                u()
            for u in c_prev:
                u()

    nc.compile()
    return nc


def _get_nc():
    if "nc" not in _CACHE:
        _CACHE["nc"] = _build()
    return _CACHE["nc"]


def kernel(**inputs):
    global LAST_RESULTS
    import ml_dtypes
    from concourse.bass_utils import run_bass_kernel_spmd

    bf = ml_dtypes.bfloat16
    x = np.asarray(inputs["x"], dtype=np.float32)
    xt = np.ascontiguousarray(x.transpose(0, 2, 1)).astype(bf)  # [B, D, S]
    Wq = np.asarray(inputs["Wq"], dtype=np.float32).astype(bf)
    Wk = np.asarray(inputs["Wk"], dtype=np.float32).astype(bf)
    Wv = np.asarray(inputs["Wv"], dtype=np.float32).astype(bf)
    Wo = np.asarray(inputs["Wo"], dtype=np.float32).astype(bf)
    bq = np.asarray(inputs["bq"], dtype=np.float32)
    bv = np.asarray(inputs["bv"], dtype=np.float32)
    bo = np.asarray(inputs["bo"], dtype=np.float32)

    def permute_w(w):  # [D, CS] -> [128, NDC, CS] (partition-major chunks)
        return np.ascontiguousarray(
            w.reshape(NDC, 128, CS).transpose(1, 0, 2)
        )

    nc = _get_nc()
    in_maps = []
    for c in range(NCORES):
        cs = slice(CS * c, CS * (c + 1))
        in_maps.append(
            {
                "xt": xt,
                "wq": permute_w(Wq[:, cs]),
                "wk": permute_w(Wk[:, cs]),
                "wv": permute_w(Wv[:, cs]),
                "wo": np.ascontiguousarray(Wo[cs, :]),
                "bq": np.ascontiguousarray(bq[cs]),
            }
        )
    res = run_bass_kernel_spmd(
        nc, in_maps, core_ids=list(range(NCORES)), trace=TRACE
    )
    LAST_RESULTS = res
    acc = np.zeros((B, S, D), dtype=np.float64)
    for c in range(NCORES):
        acc += np.asarray(res.results[c]["out"], dtype=np.float64)
    # bk drops out of softmax; bv commutes through (sum of weights = 1)
    acc += bo + bv.astype(np.float64) @ np.asarray(
        inputs["Wo"], dtype=np.float64
    )
    return acc.astype(np.float32)


# revision 24
# speedup vs baseline: 1.2982x; 1.0125x over previous
"""Multi-head attention on 8 Trainium2 NeuronCores (tensor-parallel over heads).

B=4, S=2048, D=1024, H=16 heads of DK=64. Each core owns 2 heads (a
128-channel slice of the QKV projections). x is pre-transposed on the
host to [B, D, S] so the device reads contiguous rows (no DMA transpose).

Per core, per batch b (all matmul operands bf16, fp32 PSUM accumulate):
  QT   = Wq_c^T xT + bq    [128, S]   (both heads stacked on partitions)
  KT   = Wk_c^T xT         [128, S]   (bk cancels in softmax -- dropped)
  V    = xT^T Wv_c         [S, 128] stored per-head [128, 16, 64]
  attention per (q-pair, 512-wide sub), accumulating over k-chunks kc:
    sc[128, 2, 512] psum = K_h Q_h^T for h0, h1 -- the two C=64 matmuls
       are issued adjacently so they run concurrently in the PE array
       (row tiling at partitions 0/64); sc double-buffered so ACT and PE
       ping-pong without stalls
    ex = exp(sc/8)  one ACT instr, N=1024 (amortizes the 352-cyc overhead)
    av[128, 512] psum += [V0^T ex0 ; V1^T ex1]  (col-tiled pair, M=64
       each at col positions 0/64 -- ctx^T lands in O-projection layout)
    se psum += ones^T ex   (2 col-tiled M=1 matmuls; rows 0/32 for sub0,
       64/96 for sub1, one se bank per q-pair)
  rse = 1/se (one DVE reciprocal per q-pair), broadcast via DRAM bounce,
  ctx = av * rse on DVE, out partial = ctx^T Wo_c  [S, D] bf16.
Host sums the 8 cores' partials and adds bo + bv@Wo (bv commutes through
softmax since the attention weights sum to 1).
"""

import numpy as np

B, S, D, H, DK = 4, 2048, 1024, 16, 64
NCORES = 8
CS = D // NCORES   # 128 channels (2 heads) per core
NDC = D // 128     # 8 d-chunks
NKC = S // 128     # 16 k-chunks
NST = S // 512     # 4 s-tiles
NQP = S // 1024    # 2 q-pairs

TRACE = False
LAST_RESULTS = None
_CACHE = {}


def _interleave(main, fill, start_frac=0.2):
    """Spread fill units evenly between main units (order preserved).
    No fill before start_frac of main has been emitted: the engines run
    in static order, so a fill unit whose inputs aren't ready yet would
    stall them."""
    out = []
    fi = 0
    n0 = int(len(main) * start_frac)
    for i, u in enumerate(main):
        out.append(u)
        if i < n0:
            continue
        want = (i - n0 + 1) * len(fill) // max(1, len(main) - n0)
        while fi < want:
            out.append(fill[fi])
            fi += 1
    out.extend(fill[fi:])
    return out


def _build():
    import concourse.bass as bass  # noqa: F401
    import concourse.mybir as mybir
    import concourse.tile as tile
    from concourse import bacc

    fp32 = mybir.dt.float32
    bf16 = mybir.dt.bfloat16
    AF = mybir.ActivationFunctionType

    nc = bacc.Bacc(None, target_bir_lowering=False)
    xt_d = nc.declare_dram_parameter("xt", [B, D, S], bf16, isOutput=False)
    out_d = nc.declare_dram_parameter("out", [B, S, D], bf16, isOutput=True)
    wq_d = nc.declare_dram_parameter("wq", [128, NDC, CS], bf16, isOutput=False)
    wk_d = nc.declare_dram_parameter("wk", [128, NDC, CS], bf16, isOutput=False)
    wv_d = nc.declare_dram_parameter("wv", [128, NDC, CS], bf16, isOutput=False)
    wo_d = nc.declare_dram_parameter("wo", [CS, D], bf16, isOutput=False)
    bq_d = nc.declare_dram_parameter("bq", [CS], fp32, isOutput=False)

    with tile.TileContext(nc) as tc:
        with (
            tc.tile_pool(name="consts", bufs=1) as consts,
            tc.tile_pool(name="xt", bufs=2) as xt_pool,
            tc.tile_pool(name="qk", bufs=2) as qk_pool,
            tc.tile_pool(name="vp", bufs=2) as v_pool,
            tc.tile_pool(name="exq", bufs=6) as ex_pool,
            tc.tile_pool(name="ctx", bufs=2) as ctx_pool,
            tc.tile_pool(name="avs", bufs=4) as avs_pool,
            tc.tile_pool(name="rec", bufs=2) as rec_pool,
            tc.tile_pool(name="rb", bufs=8) as rb_pool,
            tc.tile_pool(name="outp", bufs=4) as out_pool,
            tc.tile_pool(name="drp", bufs=8, space="DRAM") as dram_pool,
            tc.tile_pool(name="pssc", bufs=2, space="PSUM") as ps_sc,
            tc.tile_pool(name="psav", bufs=2, space="PSUM") as ps_av,
            tc.tile_pool(name="psse", bufs=1, space="PSUM") as ps_se,
            tc.tile_pool(name="pspj", bufs=1, space="PSUM") as ps_pj,
        ):
            wq_t = consts.tile([128, NDC, CS], bf16, tag="wq")
            wk_t = consts.tile([128, NDC, CS], bf16, tag="wk")
            wv_t = consts.tile([128, NDC, CS], bf16, tag="wv")
            wo_t = consts.tile([128, D], bf16, tag="wo")
            bq_t = consts.tile([128, 1], fp32, tag="bq")
            ones_t = consts.tile([128, 1], bf16, tag="ones")

            def load_consts():
                # wq/wk/wv arrive host-permuted as [128, NDC, CS] so these
                # are contiguous row DMAs
                nc.sync.dma_start(wq_t[:], wq_d[:])
                nc.sync.dma_start(wk_t[:], wk_d[:])
                nc.sync.dma_start(wv_t[:], wv_d[:])
                nc.sync.dma_start(wo_t[:], wo_d[:])
                nc.sync.dma_start(bq_t[:], bq_d[:].rearrange("(p o) -> p o", o=1))
                nc.gpsimd.memset(ones_t[:], 1.0)

            state = {}

            def A_xdma(bi, b):
                xT = xt_pool.tile([128, NDC, S], bf16, tag="xT")
                state[bi] = dict(xT=xT)
                xr = xt_d[b].rearrange("(c p) M -> p c M", p=128)
                # first s-half of every chunk first: the earliest
                # projection units only need s < 512
                return [
                    (lambda cch=cch, sl=sl: nc.sync.dma_start(
                        xT[:, cch, sl], xr[:, cch, sl]))
                    for sl in (slice(0, 512), slice(512, S))
                    for cch in range(NDC)
                ]

            def A_units(bi):
                st_ = state[bi]
                xT = st_["xT"]
                QT = qk_pool.tile([128, S], bf16, tag="QT")
                KT = qk_pool.tile([128, S], bf16, tag="KT")
                v0 = v_pool.tile([128, NKC, DK], bf16, tag="v0")
                v1 = v_pool.tile([128, NKC, DK], bf16, tag="v1")
                st_.update(QT=QT, KT=KT, v0=v0, v1=v1)
                # All projections run as F=128 accumulation chains packing
                # four [128,128] quarters into one psum bank -- this keeps
                # LDWEIGHTS pipelined (measured ~61ns/MM vs ~300ns at F=512)
                units = []
                for w_t, dst, is_q in ((wq_t, QT, True), (wk_t, KT, False)):
                    for g in range(4):
                        qcarry = {}
                        for j in range(4):
                            def u_p(g=g, j=j, w_t=w_t, dst=dst, is_q=is_q,
                                    qcarry=qcarry):
                                if j == 0:
                                    pq = ps_pj.tile([128, 512], fp32, tag="pj")
                                    qcarry["pq"] = pq
                                pq = qcarry["pq"]
                                sb = g * 4 + j
                                qsl = slice(j * 128, (j + 1) * 128)
                                for cch in range(NDC):
                                    nc.tensor.matmul(
                                        pq[:, qsl], w_t[:, cch, :],
                                        xT[:, cch, sb * 128 : (sb + 1) * 128],
                                        start=(cch == 0), stop=(cch == NDC - 1),
                                        skip_group_check=True,
                                    )
                                if j == 3:
                                    sl = slice(g * 512, (g + 1) * 512)
                                    if is_q:
                                        nc.vector.tensor_scalar_add(
                                            dst[:, sl], pq[:], bq_t[:]
                                        )
                                    else:
                                        nc.vector.tensor_copy(dst[:, sl], pq[:])
                            units.append(u_p)

                for g in range(4):  # groups of 4 s-blocks
                    vcarry = {}
                    for j in range(4):
                        def u_v(g=g, j=j, vcarry=vcarry):
                            if j == 0:
                                pv = ps_pj.tile([128, 512], fp32, tag="pj")
                                vcarry["pv"] = pv
                            pv = vcarry["pv"]
                            sb = g * 4 + j
                            qsl = slice(j * 128, (j + 1) * 128)
                            for cch in range(NDC):
                                nc.tensor.matmul(
                                    pv[:, qsl],
                                    xT[:, cch, sb * 128 : (sb + 1) * 128],
                                    wv_t[:, cch, :],
                                    start=(cch == 0), stop=(cch == NDC - 1),
                                    skip_group_check=True,
                                )
                            if j == 3:
                                pvv = pv[:].rearrange("p (j c) -> p j c", j=4)
                                nc.vector.tensor_copy(
                                    v0[:, g * 4 : (g + 1) * 4, :],
                                    pvv[:, :, 0:DK],
                                )
                                nc.vector.tensor_copy(
                                    v1[:, g * 4 : (g + 1) * 4, :],
                                    pvv[:, :, DK:CS],
                                )
                        units.append(u_v)
                return units

            def B_units(bi):
                """Attention for batch bi: 2 sections (one per q-pair).
                Both 512-subs of the q-pair are processed per k-chunk so
                each stationary (K_h, V_h) serves two matmuls and the 4
                sumexp matmuls run as one 4-way col-tiled pass."""
                st_ = state[bi]
                QT, KT, v0, v1 = st_["QT"], st_["KT"], st_["v0"], st_["v1"]
                ctxs = []
                sections = []
                for qp in range(NQP):
                    q0 = qp * 1024
                    ctx = ctx_pool.tile([128, 1024], bf16, tag="ctx")
                    ctxs.append(ctx)
                    carry = {}
                    units = []

                    def u_start(carry=carry):
                        av0 = ps_av.tile([128, 512], fp32, tag="av")
                        av1 = ps_av.tile([128, 512], fp32, tag="av")
                        se = ps_se.tile([128, 512], fp32, tag="se")
                        nc.vector.memset(se[:], 1.0)
                        carry.update(av=(av0, av1), se=se, ex={})
                    units.append(u_start)

                    def u_sc(kc, q0=q0, carry=carry):
                        ksl = slice(kc * 128, (kc + 1) * 128)
                        exs = []
                        for sub in range(2):
                            qsl = slice(q0 + sub * 512, q0 + (sub + 1) * 512)
                            sc = ps_sc.tile([128, 2, 512], fp32, tag="sc")
                            # h0/h1 adjacent -> concurrent row tiles (0/64);
                            # K stationaries persist across the two subs
                            nc.tensor.matmul(
                                sc[:, 0, :], KT[0:DK, ksl], QT[0:DK, qsl],
                                start=True, stop=True,
                            )
                            nc.tensor.matmul(
                                sc[:, 1, :], KT[DK:CS, ksl], QT[DK:CS, qsl],
                                start=True, stop=True,
                            )
                            ex = ex_pool.tile([128, 2, 512], bf16, tag="ex")
                            nc.scalar.activation(
                                ex[:], sc[:], AF.Exp, scale=0.125
                            )
                            exs.append(ex)
                        carry["ex"][kc] = exs

                    def u_av(kc, carry=carry):
                        ex0, ex1 = carry["ex"].pop(kc)
                        av0, av1 = carry["av"]
                        se = carry["se"]
                        first, last = kc == 0, kc == NKC - 1
                        # col-tiled pairs: h0 -> rows 0-63, h1 -> 64-127;
                        # V stationaries persist across the two subs
                        for ex, av in ((ex0, av0), (ex1, av1)):
                            nc.tensor.matmul(
                                av[0:DK, :], v0[:, kc, :], ex[:, 0, :],
                                start=first, stop=last, skip_group_check=True,
                            )
                            nc.tensor.matmul(
                                av[DK:CS, :], v1[:, kc, :], ex[:, 1, :],
                                start=first, stop=last, skip_group_check=True,
                            )
                        # sumexp: 4-way col-tiled pass, rows (sub, h) ->
                        # 0:(s0,h0) 32:(s0,h1) 64:(s1,h0) 96:(s1,h1)
                        for qi, ex in enumerate((ex0, ex0, ex1, ex1)):
                            p = 32 * qi
                            nc.tensor.matmul(
                                se[p : p + 1, :], ones_t[:], ex[:, qi % 2, :],
                                start=first, stop=last,
                                skip_group_check=True,
                                tile_position=(0, p),
                            )

                    for kc in range(NKC):
                        def u_kc(kc=kc, u_sc=u_sc, u_av=u_av):
                            u_sc(kc)
                            if kc > 0:
                                u_av(kc - 1)
                            if kc == NKC - 1:
                                u_av(kc)
                        units.append(u_kc)

                    def u_end(carry=carry):
                        # drain the av/se psum banks quickly; the rest of
                        # the normalize chain (u_norm) is emitted a few
                        # units into the next section so the 3 DVE ops +
                        # DRAM bounce don't block the pipeline here
                        av = carry["av"]
                        avss = []
                        for s in range(2):
                            avs = avs_pool.tile([128, 512], fp32, tag="avs")
                            nc.vector.tensor_copy(avs[:], av[s][:])
                            avss.append(avs)
                        rse = rec_pool.tile([128, 512], fp32, tag="rse")
                        nc.vector.reciprocal_approx_fast(rse[:], carry["se"][:])
                        carry.update(avss=avss, rse=rse)
                    units.append(u_end)

                    def u_norm(carry=carry, ctx=ctx):
                        rse = carry["rse"]
                        for s in range(2):
                            # rb rows 0-63 <- 1/se(h0), 64-127 <- 1/se(h1)
                            rb = rb_pool.tile([128, 512], fp32, tag="rb")
                            for h in range(2):
                                dr = dram_pool.tile([1, 512], fp32, tag="dr")
                                nc.sync.dma_start(
                                    dr[:],
                                    rse[64 * s + 32 * h : 64 * s + 32 * h + 1, :],
                                )
                                nc.sync.dma_start(
                                    rb[h * DK : (h + 1) * DK, :],
                                    dr[:].partition_broadcast(DK),
                                )
                            ssl = slice(s * 512, (s + 1) * 512)
                            nc.vector.tensor_mul(
                                ctx[:, ssl], carry["avss"][s][:], rb[:]
                            )
                    sections.append((units, u_norm))
                st_["ctx"] = ctxs
                return sections

            def C_units(bi, b, qp):
                ctx = state[bi]["ctx"][qp]
                units = []
                for j in range(8):
                    ocarry = {}
                    for half in range(2):
                        def u_o(j=j, half=half, ocarry=ocarry):
                            sb = qp * 8 + j
                            lsl = slice(j * 128, (j + 1) * 128)
                            if half == 0:
                                ot = out_pool.tile([128, D], bf16, tag="ot")
                                ocarry["ot"] = ot
                            ot = ocarry["ot"]
                            osl = slice(half * 512, (half + 1) * 512)
                            po = ps_pj.tile([128, 512], fp32, tag="pj")
                            for oc in range(4):
                                nc.tensor.matmul(
                                    po[:, oc * 128 : (oc + 1) * 128],
                                    ctx[:, lsl],
                                    wo_t[:, half * 512 + oc * 128 :
                                         half * 512 + (oc + 1) * 128],
                                    start=True, stop=True,
                                    skip_group_check=True,
                                )
                            nc.vector.tensor_copy(ot[:, osl], po[:])
                            if half == 1:
                                nc.sync.dma_start(
                                    out_d[b, sb * 128 : (sb + 1) * 128, :],
                                    ot[:],
                                )
                        units.append(u_o)
                return units

            # ---- software pipeline over batches ----
            for u in A_xdma(0, 0):
                u()
            load_consts()
            # batch 0 ramp: emit only the projection prefix attention
            # needs immediately (Q g0-1, K g0, V g0); the rest interleaves
            # into batch 0's own attention, ordered by first use.
            a0 = A_units(0)
            a_cur = []
            for u in a0[0:8] + a0[16:20] + a0[32:36]:
                u()
            a0_rest = []
            for g in range(1, 4):
                a0_rest += a0[16 + 4 * g : 20 + 4 * g]  # K g
                a0_rest += a0[32 + 4 * g : 36 + 4 * g]  # V g
            a0_rest += a0[8:16]  # Q g2, g3
            c_prev = []   # C units of (bi-1, qp1)
            norm_prev = []  # deferred normalize of (bi-1, qp1)
            for bi in range(B):
                for u in a_cur:
                    u()
                (sec0, norm0), (sec1, norm1) = B_units(bi)
                if bi + 1 < B:
                    for u in A_xdma(bi + 1, bi + 1):
                        u()
                    a_next = A_units(bi + 1)
                else:
                    a_next = []
                half = len(a_next) // 2
                # Fill order: batch-0 leftovers / deferred normalize first
                # (their consumers sit later in the list), then next
                # batch's projections, then the freshest out-projection.
                for u in _interleave(
                    sec0, a0_rest + norm_prev + a_next[:half] + c_prev, 0.08
                ):
                    u()
                a0_rest = []
                c_q0 = C_units(bi, bi, 0)
                for u in _interleave(
                    sec1, [norm0] + a_next[half:] + c_q0, 0.08
                ):
                    u()
                c_prev = C_units(bi, bi, 1)
                norm_prev = [norm1]
                a_cur = []
            for u in norm_prev:
                u()
            for u in c_prev:
                u()

    nc.compile()
    return nc


def _get_nc():
    if "nc" not in _CACHE:
        _CACHE["nc"] = _build()
    return _CACHE["nc"]


def kernel(**inputs):
    global LAST_RESULTS
    import ml_dtypes
    from concourse.bass_utils import run_bass_kernel_spmd

    bf = ml_dtypes.bfloat16
    x = np.asarray(inputs["x"], dtype=np.float32)
    xt = np.ascontiguousarray(x.transpose(0, 2, 1)).astype(bf)  # [B, D, S]
    Wq = np.asarray(inputs["Wq"], dtype=np.float32).astype(bf)
    Wk = np.asarray(inputs["Wk"], dtype=np.float32).astype(bf)
    Wv = np.asarray(inputs["Wv"], dtype=np.float32).astype(bf)
    Wo = np.asarray(inputs["Wo"], dtype=np.float32).astype(bf)
    bq = np.asarray(inputs["bq"], dtype=np.float32)
    bv = np.asarray(inputs["bv"], dtype=np.float32)
    bo = np.asarray(inputs["bo"], dtype=np.float32)

    def permute_w(w):  # [D, CS] -> [128, NDC, CS] (partition-major chunks)
        return np.ascontiguousarray(
            w.reshape(NDC, 128, CS).transpose(1, 0, 2)
        )

    nc = _get_nc()
    in_maps = []
    for c in range(NCORES):
        cs = slice(CS * c, CS * (c + 1))
        in_maps.append(
            {
                "xt": xt,
                "wq": permute_w(Wq[:, cs]),
                "wk": permute_w(Wk[:, cs]),
                "wv": permute_w(Wv[:, cs]),
                "wo": np.ascontiguousarray(Wo[cs, :]),
                "bq": np.ascontiguousarray(bq[cs]),
            }
        )
    res = run_bass_kernel_spmd(
        nc, in_maps, core_ids=list(range(NCORES)), trace=TRACE
    )
    LAST_RESULTS = res
    acc = np.zeros((B, S, D), dtype=np.float64)
    for c in range(NCORES):
        acc += np.asarray(res.results[c]["out"], dtype=np.float64)
    # bk drops out of softmax; bv commutes through (sum of weights = 1)
    acc += bo + bv.astype(np.float64) @ np.asarray(
        inputs["Wo"], dtype=np.float64
    )
    return acc.astype(np.float32)
